# revision 9
# baseline (speedup 1.0000x reference)
"""Trainium2 Bass kernel for BlockDiagMNIST MLP.

Reference computation (all fp32):
    h  = relu(x @ W1.T + b1)          x:[B,784], W1:[4096,784]    -> [B,4096]
    yb = blockdiag(h, Wb)             Wb:[128,32,32] (h2[b, 32n+o] = sum_k h[b,32n+k] Wb[n,o,k])
    h2 = relu(yb + bb)
    out = h2 @ W3.T + b3              W3:[10,4096]                -> [B,10]

Strategy: pure data-parallel over batch (B=32768 -> 4096 rows/core on 8 cores),
weights replicated.  All matmuls in bf16 (fp32 PSUM accumulation, fp32 biases).
On-chip layout is transposed ("hidden on partitions"): we compute
hT = W1 @ x.T per 512-column batch window.

Layer 2 (block-diagonal) runs as packs of eight concurrent 32x32 PE-array
tiles (tile_position row x col grid): tile (32r, 32c) applies one diagonal
block to h-slice [32r:32r+32] of m-tile (4g+c), writing psum partitions
[32c:32c+32].  Two packs (block rows 0,1 then 2,3) cover a 4-m-tile group
using 2 PSUM banks; outputs land hidden-permuted, which the host-side bb/W3
packing compensates.

Layer 3 (M=10) runs as four concurrent column-tiles: K-chunk t accumulates
into psum partitions [32*(t%4) : +10]; a final DVE pass sums the four bands
(cross-quadrant reads) and adds b3.

Host-side prep (free -- not on the device timeline): transpose + bf16-cast of
x and weights, block/bias/W3 permutation packing.
"""

import numpy as np
import ml_dtypes

B = 32768
IN_DIM = 784
HIDDEN = 4096
BLOCK = 32
NUM_BLOCKS = 128
OUT_DIM = 10
NCORES = 8
BC = B // NCORES          # batch rows per core (4096)
WN = 512                  # batch-window columns (one matmul free-dim)
K1 = 6                    # full 128-row K-chunks for layer 1 (features 0..767)
KL = 16                   # leftover K rows (features 768..783), row-group packed
NM = HIDDEN // 128        # 32 hidden tiles per window
NGW = NM // 4             # 8 groups of 4 m-tiles per window
NBAND = 4                 # layer-3 column-tile bands

BF16 = ml_dtypes.bfloat16

_PROGRAM_CACHE = {}


def _build_program(bc=BC):
    """Build (and bacc-compile) the per-core Bass program. bc = batch cols/core."""
    import concourse.mybir as mybir
    import concourse.tile as tile
    from concourse import bacc

    nw = bc // WN
    f32, bf16 = mybir.dt.float32, mybir.dt.bfloat16

    nc = bacc.Bacc("TRN2", target_bir_lowering=False, debug=False)

    # x / W1 stored pre-swizzled for per-partition contiguity (3 KB runs):
    # xT[p, w, k, b], W1T[p, j, k, m]
    xT = nc.dram_tensor("xT", [128, nw, K1, WN], bf16, kind="ExternalInput").ap()
    xL = nc.dram_tensor("xL", [128, bc], bf16, kind="ExternalInput").ap()
    w1t = nc.dram_tensor(
        "W1T", [128, NM // 4, K1, 4 * 128], bf16, kind="ExternalInput"
    ).ap()
    w1l = nc.dram_tensor("W1L", [128, HIDDEN], bf16, kind="ExternalInput").ap()
    # Wb2: block (4*(4g+c)+r).T at [32r:32r+32, g*128+c*32 : +32]
    wb2 = nc.dram_tensor("Wb2", [128, NGW * 128], bf16, kind="ExternalInput").ap()
    # W3P: chunk t=4g+band -> [128, 10] at cols t*10; rows permuted to match the
    # pack layout (partition 32c+o of band tile <-> hidden 128*(4g+c)+32*band+o)
    w3p = nc.dram_tensor("W3P", [128, NM * OUT_DIM], bf16, kind="ExternalInput").ap()
    # biases packed into one tensor: cols 0..NM-1 = b1, NM..NM+127 = bbP
    # (col NM+4g+band, permuted like the L2 pack output), col NM+128 = b3
    NBC = NM + 128 + 1
    bcat = nc.dram_tensor("bcat", [128, NBC], f32, kind="ExternalInput").ap()
    outT = nc.dram_tensor("outT", [OUT_DIM, bc], f32, kind="ExternalOutput").ap()

    Relu = mybir.ActivationFunctionType.Relu
    Add = mybir.AluOpType.add
    Max = mybir.AluOpType.max

    MB = 4          # W1T column-block = MB m-tiles (DMA granularity for overlap)
    NJ = NM // MB   # 8 column blocks

    with tile.TileContext(nc) as tc:
        with (
            tc.tile_pool(name="const", bufs=1) as cpool,
            tc.tile_pool(name="xin", bufs=3) as xpool,
            tc.tile_pool(name="hbuf", bufs=10) as hpool,
            tc.tile_pool(name="h2buf", bufs=10) as h2pool,
            tc.tile_pool(name="obuf", bufs=4) as opool,
            tc.tile_pool(name="ps1", bufs=5, space="PSUM") as ps1,
            tc.tile_pool(name="ps2", bufs=2, space="PSUM") as ps2,
            tc.tile_pool(name="ps3", bufs=1, space="PSUM") as ps3,
        ):
            # HAM warmup on an un-DMA'd (garbage) SBUF tile: no data
            # dependency, so the PE clock gate starts ramping at t~0 and the
            # warmup stream covers the DMA-launch latency until real x/W1
            # data lands. Values are irrelevant (dummy psum, never read).
            gsb = cpool.tile([128, WN], bf16, name="warm_src")
            nc.vector.memset(gsb[:], 0)
            pw = ps2.tile([65, WN], f32, tag="p2", name="pwarm")
            for _ in range(20):
                nc.tensor.matmul(
                    pw[:, 0:65], gsb[:, 0:65], gsb[:, 0:65],
                    start=True, stop=True,
                )
            for _ in range(36):
                nc.tensor.matmul(
                    pw[:], gsb[:, 0:65], gsb[:],
                    start=True, stop=True,
                )

            def load_xt(w):
                """Per-window x tiles: two k-half DMAs (parallel queues, so the
                first window's data lands sooner) + the leftover rows."""
                KH = K1 // 2
                ta = xpool.tile([128, KH, WN], bf16, tag="xta", name=f"xta_{w}")
                nc.sync.dma_start(ta[:], xT[:, w, 0:KH, :])
                tb = xpool.tile([128, K1 - KH, WN], bf16, tag="xtb", name=f"xtb_{w}")
                nc.sync.dma_start(tb[:], xT[:, w, KH:K1, :])
                tl = xpool.tile([128, WN], bf16, tag="xl", name=f"xl_{w}")
                if w > 0:
                    nc.sync.dma_start(tl[:], xL[:, w * WN:(w + 1) * WN])
                return (ta, tb), tl

            # Window-0 x tile before everything else so PE starts early.
            xts = {0: load_xt(0)}

            # Small constants (ACT/DVE need them by the first relu).
            bc_sb = cpool.tile([128, NBC], f32)
            nc.sync.dma_start(bc_sb[:], bcat)
            b1_sb = bc_sb[:, 0:NM]
            bb_sb = bc_sb[:, NM:NM + 128]
            b3_sb = bc_sb[0:OUT_DIM, NM + 128:NBC]

            # W1T as NJ column blocks [128, K1, MB*128], one 3D DMA each; the
            # first block (m-tiles 0..MB-1) lands ahead of everything else.
            w1t_t = [None] * NJ
            KH = K1 // 2
            for j in range(NJ):
                ta = cpool.tile([128, KH, MB * 128], bf16, name=f"w1ta_{j}")
                nc.sync.dma_start(ta[:], w1t[:, j, 0:KH, :])
                tb = cpool.tile([128, K1 - KH, MB * 128], bf16, name=f"w1tb_{j}")
                nc.sync.dma_start(tb[:], w1t[:, j, KH:K1, :])
                w1t_t[j] = (ta, tb)
                if j == 0:
                    # window-0 leftover rows: needed only at the end of the
                    # first group, so load after the critical j=0 weights
                    nc.sync.dma_start(xts[0][1][:], xL[:, 0:WN])
                    w1l_sb = cpool.tile([128, HIDDEN], bf16)
                    nc.sync.dma_start(w1l_sb[:], w1l)
                    wb2_sb = cpool.tile([128, NGW * 128], bf16)
                    nc.sync.dma_start(wb2_sb[:], wb2)
                    w3p_sb = cpool.tile([128, NM * OUT_DIM], bf16)
                    nc.sync.dma_start(w3p_sb[:], w3p)

            # Software pipeline over G (global group index = window*NGW + g):
            #   L1(G) fulls interleaved with L2 packs of G-1 | L1 quad |
            #   L3 quad of G-2.
            NGTOT = nw * NGW
            pos = {}    # window -> psum accumulator for layer 3
            hs = {}     # G -> [4 h tiles]
            p1s_live = {}   # G -> [4 psum tiles] for the quad
            h2s = {}    # (G, band) -> h2 tile

            def emit_l1_fulls(G, js):
                w, g = divmod(G, NGW)
                if g == 0 and js[0] == 0 and w not in xts:
                    xts[w] = load_xt(w)
                xt, _ = xts[w]
                ps = p1s_live.setdefault(G, [None] * 4)
                KH = K1 // 2
                for j in js:
                    m = 4 * g + j
                    p1 = ps1.tile([128, WN], f32, tag="p1", name=f"p1_{G}_{j}")
                    for k in range(K1):
                        hi = k >= KH
                        nc.tensor.matmul(
                            p1[:],
                            w1t_t[m // MB][hi][:, k - KH * hi, (m % MB) * 128:(m % MB + 1) * 128],
                            xt[hi][:, k - KH * hi, :],
                            start=(k == 0),
                            stop=False,
                        )
                    ps[j] = p1

            def emit_l1_quad(G):
                w, g = divmod(G, NGW)
                _, xl = xts[w]
                ps = p1s_live.pop(G)
                for j in range(4):
                    m = 4 * g + j
                    nc.tensor.matmul(
                        ps[j][:],
                        w1l_sb[32 * j:32 * j + KL, m * 128:(m + 1) * 128],
                        xl[32 * j:32 * j + KL, :],
                        start=False,
                        stop=True,
                        tile_position=(32 * j, 0),
                    )
                hts = []
                for j in range(4):
                    m = 4 * g + j
                    h = hpool.tile([128, WN], bf16, tag="h", name=f"h_{G}_{j}")
                    nc.scalar.activation(h[:], ps[j][:], Relu, bias=b1_sb[:, m:m + 1])
                    hts.append(h)
                hs[G] = hts

            def emit_l2_pack(G, half):
                """8 concurrent 32x32 tiles: block rows (2*half, 2*half+1) of
                the 4 m-tiles of group G, into 2 psum banks."""
                w, g = divmod(G, NGW)
                hts = hs[G]
                pk = [
                    ps2.tile([128, WN], f32, tag="p2", name=f"p2_{G}_{half}_{i}")
                    for i in range(2)
                ]
                for c in range(4):
                    for i in range(2):
                        r = 2 * half + i
                        nc.tensor.matmul(
                            pk[i][32 * c:32 * c + 32, :],
                            wb2_sb[32 * r:32 * r + 32,
                                   g * 128 + c * 32:g * 128 + c * 32 + 32],
                            hts[c][32 * r:32 * r + 32, :],
                            start=True,
                            stop=True,
                            tile_position=(32 * r, 32 * c),
                        )
                if half == 1:
                    hs.pop(G)
                for i in range(2):
                    band = 2 * half + i
                    h2 = h2pool.tile([128, WN], bf16, tag="h2",
                                     name=f"h2_{G}_{band}")
                    nc.vector.tensor_scalar(
                        h2[:], pk[i][:], bb_sb[:, 4 * g + band:4 * g + band + 1],
                        0.0, Add, Max,
                    )
                    h2s[(G, band)] = h2

            def emit_l3_quad(G):
                w, g = divmod(G, NGW)
                if g == 0:
                    pos[w] = ps3.tile([128, WN], f32, tag="po", name=f"po_{w}")
                po = pos[w]
                for band in range(NBAND):
                    t = 4 * g + band
                    nc.tensor.matmul(
                        po[32 * band:32 * band + OUT_DIM, :],
                        w3p_sb[:, t * OUT_DIM:(t + 1) * OUT_DIM],
                        h2s.pop((G, band))[:],
                        start=(g == 0),
                        stop=(g == NGW - 1),
                        skip_group_check=True,
                        tile_position=(0, 32 * band),
                    )
                if g == NGW - 1:
                    # band merge: only one PSUM operand per DVE op, so chain
                    # SBUF accumulators across the four bands (+ b3).
                    po = pos.pop(w)
                    t1 = opool.tile([OUT_DIM, WN], f32, tag="t1", name=f"t1_{w}")
                    nc.vector.tensor_scalar_add(t1[:], po[0:OUT_DIM, :], b3_sb[:])
                    t2 = opool.tile([OUT_DIM, WN], f32, tag="t2", name=f"t2_{w}")
                    nc.vector.tensor_tensor(
                        t2[:], t1[:], po[32:32 + OUT_DIM, :], Add
                    )
                    t3 = opool.tile([OUT_DIM, WN], f32, tag="t3", name=f"t3_{w}")
                    nc.vector.tensor_tensor(
                        t3[:], t2[:], po[64:64 + OUT_DIM, :], Add
                    )
                    ot = opool.tile([OUT_DIM, WN], f32, tag="ot", name=f"ot_{w}")
                    nc.vector.tensor_tensor(
                        ot[:], t3[:], po[96:96 + OUT_DIM, :], Add
                    )
                    nc.sync.dma_start(outT[:, w * WN:(w + 1) * WN], ot[:])

            for G in range(NGTOT + 2):
                if G < NGTOT:
                    emit_l1_fulls(G, (0, 1))
                if 1 <= G <= NGTOT:
                    emit_l2_pack(G - 1, 0)
                if G < NGTOT:
                    emit_l1_fulls(G, (2, 3))
                if 1 <= G <= NGTOT:
                    emit_l2_pack(G - 1, 1)
                if G < NGTOT:
                    emit_l1_quad(G)
                if G >= 2:
                    emit_l3_quad(G - 2)

    nc.compile()
    return nc


def _get_program(bc=BC):
    if bc not in _PROGRAM_CACHE:
        _PROGRAM_CACHE[bc] = _build_program(bc)
    return _PROGRAM_CACHE[bc]


def _prep_weights(W1, b1, Wb, bb, W3, b3):
    """Host-side packing of replicated weights into device layouts."""
    W1 = np.asarray(W1, dtype=np.float32)
    Wb = np.asarray(Wb, dtype=np.float32)
    W3 = np.asarray(W3, dtype=np.float32)
    bb = np.asarray(bb, dtype=np.float32)

    # W1T [128, NJ, K1, 512]: W1T[p, j, k, m] = W1.T[128k+p, 512j+m] -- the
    # per-partition-contiguous swizzle (3 KB DMA runs). W1L [128, 4096] holds
    # the 16 leftover feature rows replicated at partition offsets 0/32/64/96
    # for the row-group-packed leftover matmuls.
    NJ = NM // 4
    W1T = np.ascontiguousarray(
        W1.T[:K1 * 128].reshape(K1, 128, NJ, 512).transpose(1, 2, 0, 3)
    ).astype(BF16)
    W1L = np.zeros((128, HIDDEN), dtype=BF16)
    lo = W1.T[K1 * 128:IN_DIM].astype(BF16)
    for j in range(4):
        W1L[32 * j:32 * j + KL] = lo

    # Wb2 [128, NGW*128]: block (4*(4g+c)+r).T at [32r:+32, g*128+c*32:+32].
    Wb2 = np.zeros((128, NGW * 128), dtype=BF16)
    for g in range(NGW):
        for c in range(4):
            for r in range(4):
                blk = Wb[4 * (4 * g + c) + r].T.astype(BF16)  # [k, o]
                Wb2[32 * r:32 * r + 32,
                    g * 128 + c * 32:g * 128 + c * 32 + 32] = blk

    # Pack-output permutation: h2 band tile (g, band) partition 32c+o holds
    # hidden dim 128*(4g+c) + 32*band + o.
    # W3P [128, NM*10]: chunk t=4g+band at cols [t*10:+10]; W3P[p, t*10+o] =
    # W3[o, hid(g, band, p)].
    W3P = np.zeros((128, NM * OUT_DIM), dtype=BF16)
    bbP = np.zeros((128, 128), dtype=np.float32)
    for g in range(NGW):
        for band in range(4):
            t = 4 * g + band
            for c in range(4):
                hid0 = 128 * (4 * g + c) + 32 * band
                W3P[32 * c:32 * c + 32, t * OUT_DIM:(t + 1) * OUT_DIM] = (
                    W3[:, hid0:hid0 + 32].T.astype(BF16)
                )
                bbP[32 * c:32 * c + 32, t] = bb[hid0:hid0 + 32]

    NBC = NM + 128 + 1
    bcat = np.zeros((128, NBC), dtype=np.float32)
    bcat[:, 0:NM] = np.asarray(b1, np.float32).reshape(NM, 128).T
    bcat[:, NM:NM + 128] = bbP
    bcat[0:OUT_DIM, NM + 128] = np.asarray(b3, np.float32)
    return dict(W1T=W1T, W1L=W1L, Wb2=Wb2, W3P=W3P, bcat=bcat)


def _prep_x_shard(x, c, ncores=NCORES, bc=BC):
    xs = np.asarray(x[c * bc:(c + 1) * bc], dtype=np.float32).T.astype(BF16)  # [784, bc]
    nw = bc // WN
    # xT [128, nw, K1, WN]: xT[p, w, k, b] = x.T[128k+p, 512w+b]
    xT = np.ascontiguousarray(
        xs[:K1 * 128].reshape(K1, 128, nw, WN).transpose(1, 2, 0, 3)
    )
    xLs = np.zeros((128, bc), dtype=BF16)
    for j in range(4):
        xLs[32 * j:32 * j + KL] = xs[K1 * 128:IN_DIM]
    return xT, xLs


def run(x, W1, b1, Wb, bb, W3, b3, trace=False, tmpdir=None):
    """Run on 8 cores; returns (out [B,10] fp32, BassKernelResults)."""
    from concourse.bass_utils import run_bass_kernel_spmd

    nc = _get_program()
    wmap = _prep_weights(W1, b1, Wb, bb, W3, b3)
    in_maps = []
    for c in range(NCORES):
        m = dict(wmap)
        m["xT"], m["xL"] = _prep_x_shard(np.asarray(x), c)
        in_maps.append(m)

    res = run_bass_kernel_spmd(
        nc, in_maps, core_ids=list(range(NCORES)), trace=trace, tmpdir=tmpdir
    )
    out = np.concatenate(
        [np.asarray(r["outT"]).T for r in res.results], axis=0
    ).astype(np.float32)
    return out, res


def kernel(x, W1, b1, Wb, bb, W3, b3):
    out, _ = run(x, W1, b1, Wb, bb, W3, b3, trace=False)
    return out


# revision 10
# speedup vs baseline: 1.0019x; 1.0019x over previous
"""Trainium2 Bass kernel for BlockDiagMNIST MLP.

Reference computation (all fp32):
    h  = relu(x @ W1.T + b1)          x:[B,784], W1:[4096,784]    -> [B,4096]
    yb = blockdiag(h, Wb)             Wb:[128,32,32] (h2[b, 32n+o] = sum_k h[b,32n+k] Wb[n,o,k])
    h2 = relu(yb + bb)
    out = h2 @ W3.T + b3              W3:[10,4096]                -> [B,10]

Strategy: pure data-parallel over batch (B=32768 -> 4096 rows/core on 8 cores),
weights replicated.  All matmuls in bf16 (fp32 PSUM accumulation, fp32 biases).
On-chip layout is transposed ("hidden on partitions"): we compute
hT = W1 @ x.T per 512-column batch window.

Layer 2 (block-diagonal) runs as packs of eight concurrent 32x32 PE-array
tiles (tile_position row x col grid): tile (32r, 32c) applies one diagonal
block to h-slice [32r:32r+32] of m-tile (4g+c), writing psum partitions
[32c:32c+32].  Two packs (block rows 0,1 then 2,3) cover a 4-m-tile group
using 2 PSUM banks; outputs land hidden-permuted, which the host-side bb/W3
packing compensates.

Layer 3 (M=10) runs as four concurrent column-tiles: K-chunk t accumulates
into psum partitions [32*(t%4) : +10]; a final DVE pass sums the four bands
(cross-quadrant reads) and adds b3.

Host-side prep (free -- not on the device timeline): transpose + bf16-cast of
x and weights, block/bias/W3 permutation packing.
"""

import numpy as np
import ml_dtypes

B = 32768
IN_DIM = 784
HIDDEN = 4096
BLOCK = 32
NUM_BLOCKS = 128
OUT_DIM = 10
NCORES = 8
BC = B // NCORES          # batch rows per core (4096)
WN = 512                  # batch-window columns (one matmul free-dim)
K1 = 6                    # full 128-row K-chunks for layer 1 (features 0..767)
KL = 16                   # leftover K rows (features 768..783), row-group packed
NM = HIDDEN // 128        # 32 hidden tiles per window
NGW = NM // 4             # 8 groups of 4 m-tiles per window
NBAND = 4                 # layer-3 column-tile bands

BF16 = ml_dtypes.bfloat16

_PROGRAM_CACHE = {}


def _build_program(bc=BC):
    """Build (and bacc-compile) the per-core Bass program. bc = batch cols/core."""
    import concourse.mybir as mybir
    import concourse.tile as tile
    from concourse import bacc

    nw = bc // WN
    f32, bf16 = mybir.dt.float32, mybir.dt.bfloat16

    nc = bacc.Bacc("TRN2", target_bir_lowering=False, debug=False)

    # x / W1 stored pre-swizzled for per-partition contiguity (3 KB runs):
    # xT[p, w, k, b], W1T[p, j, k, m]
    xT = nc.dram_tensor("xT", [128, nw, K1, WN], bf16, kind="ExternalInput").ap()
    xL = nc.dram_tensor("xL", [128, bc], bf16, kind="ExternalInput").ap()
    w1t = nc.dram_tensor(
        "W1T", [128, NM // 4, K1, 4 * 128], bf16, kind="ExternalInput"
    ).ap()
    w1l = nc.dram_tensor("W1L", [128, HIDDEN], bf16, kind="ExternalInput").ap()
    # Wb2: block (4*(4g+c)+r).T at [32r:32r+32, g*128+c*32 : +32]
    wb2 = nc.dram_tensor("Wb2", [128, NGW * 128], bf16, kind="ExternalInput").ap()
    # W3P: chunk t=4g+band -> [128, 10] at cols t*10; rows permuted to match the
    # pack layout (partition 32c+o of band tile <-> hidden 128*(4g+c)+32*band+o)
    w3p = nc.dram_tensor("W3P", [128, NM * OUT_DIM], bf16, kind="ExternalInput").ap()
    # biases packed into one tensor: cols 0..NM-1 = b1, NM..NM+127 = bbP
    # (col NM+4g+band, permuted like the L2 pack output), col NM+128 = b3
    NBC = NM + 128 + 1
    bcat = nc.dram_tensor("bcat", [128, NBC], f32, kind="ExternalInput").ap()
    outT = nc.dram_tensor("outT", [OUT_DIM, bc], f32, kind="ExternalOutput").ap()

    Relu = mybir.ActivationFunctionType.Relu
    Add = mybir.AluOpType.add
    Max = mybir.AluOpType.max

    MB = 4          # W1T column-block = MB m-tiles (DMA granularity for overlap)
    NJ = NM // MB   # 8 column blocks

    with tile.TileContext(nc) as tc:
        with (
            tc.tile_pool(name="const", bufs=1) as cpool,
            tc.tile_pool(name="xin", bufs=3) as xpool,
            tc.tile_pool(name="hbuf", bufs=10) as hpool,
            tc.tile_pool(name="h2buf", bufs=10) as h2pool,
            tc.tile_pool(name="obuf", bufs=4) as opool,
            tc.tile_pool(name="ps1", bufs=5, space="PSUM") as ps1,
            tc.tile_pool(name="ps2", bufs=2, space="PSUM") as ps2,
            tc.tile_pool(name="ps3", bufs=1, space="PSUM") as ps3,
        ):
            # HAM warmup on an un-DMA'd (garbage) SBUF tile: no data
            # dependency, so the PE clock gate starts ramping at t~0 and the
            # warmup stream covers the DMA-launch latency until real x/W1
            # data lands. Values are irrelevant (dummy psum, never read).
            gsb = cpool.tile([128, WN], bf16, name="warm_src")
            nc.vector.memset(gsb[:], 0)
            pw = ps2.tile([65, WN], f32, tag="p2", name="pwarm")
            for _ in range(20):
                nc.tensor.matmul(
                    pw[:, 0:65], gsb[:, 0:65], gsb[:, 0:65],
                    start=True, stop=True,
                )
            for _ in range(5):
                nc.tensor.matmul(
                    pw[:], gsb[:, 0:65], gsb[:],
                    start=True, stop=True,
                )

            def load_xt(w):
                """Per-window x tiles: two k-half DMAs (parallel queues, so the
                first window's data lands sooner) + the leftover rows."""
                KH = K1 // 2
                ta = xpool.tile([128, KH, WN], bf16, tag="xta", name=f"xta_{w}")
                nc.sync.dma_start(ta[:], xT[:, w, 0:KH, :])
                tb = xpool.tile([128, K1 - KH, WN], bf16, tag="xtb", name=f"xtb_{w}")
                nc.sync.dma_start(tb[:], xT[:, w, KH:K1, :])
                tl = xpool.tile([128, WN], bf16, tag="xl", name=f"xl_{w}")
                if w > 0:
                    nc.sync.dma_start(tl[:], xL[:, w * WN:(w + 1) * WN])
                return (ta, tb), tl

            # Window-0 x tile before everything else so PE starts early.
            xts = {0: load_xt(0)}

            # Small constants (ACT/DVE need them by the first relu).
            bc_sb = cpool.tile([128, NBC], f32)
            nc.sync.dma_start(bc_sb[:], bcat)
            b1_sb = bc_sb[:, 0:NM]
            bb_sb = bc_sb[:, NM:NM + 128]
            b3_sb = bc_sb[0:OUT_DIM, NM + 128:NBC]

            # W1T as NJ column blocks [128, K1, MB*128], one 3D DMA each; the
            # first block (m-tiles 0..MB-1) lands ahead of everything else.
            w1t_t = [None] * NJ
            KH = K1 // 2
            for j in range(NJ):
                ta = cpool.tile([128, KH, MB * 128], bf16, name=f"w1ta_{j}")
                nc.sync.dma_start(ta[:], w1t[:, j, 0:KH, :])
                tb = cpool.tile([128, K1 - KH, MB * 128], bf16, name=f"w1tb_{j}")
                nc.sync.dma_start(tb[:], w1t[:, j, KH:K1, :])
                w1t_t[j] = (ta, tb)
                if j == 0:
                    # window-0 leftover rows: needed only at the end of the
                    # first group, so load after the critical j=0 weights
                    nc.sync.dma_start(xts[0][1][:], xL[:, 0:WN])
                    w1l_sb = cpool.tile([128, HIDDEN], bf16)
                    nc.sync.dma_start(w1l_sb[:], w1l)
                    wb2_sb = cpool.tile([128, NGW * 128], bf16)
                    nc.sync.dma_start(wb2_sb[:], wb2)
                    w3p_sb = cpool.tile([128, NM * OUT_DIM], bf16)
                    nc.sync.dma_start(w3p_sb[:], w3p)

            # Software pipeline over G (global group index = window*NGW + g):
            #   L1(G) fulls interleaved with L2 packs of G-1 | L1 quad |
            #   L3 quad of G-2.
            NGTOT = nw * NGW
            pos = {}    # window -> psum accumulator for layer 3
            hs = {}     # G -> [4 h tiles]
            p1s_live = {}   # G -> [4 psum tiles] for the quad
            h2s = {}    # (G, band) -> h2 tile

            def emit_l1_fulls(G, js):
                w, g = divmod(G, NGW)
                if g == 0 and js[0] == 0 and w not in xts:
                    xts[w] = load_xt(w)
                xt, _ = xts[w]
                ps = p1s_live.setdefault(G, [None] * 4)
                KH = K1 // 2
                for j in js:
                    m = 4 * g + j
                    p1 = ps1.tile([128, WN], f32, tag="p1", name=f"p1_{G}_{j}")
                    for k in range(K1):
                        hi = k >= KH
                        nc.tensor.matmul(
                            p1[:],
                            w1t_t[m // MB][hi][:, k - KH * hi, (m % MB) * 128:(m % MB + 1) * 128],
                            xt[hi][:, k - KH * hi, :],
                            start=(k == 0),
                            stop=False,
                        )
                    ps[j] = p1

            def emit_l1_quad(G):
                w, g = divmod(G, NGW)
                _, xl = xts[w]
                ps = p1s_live.pop(G)
                for j in range(4):
                    m = 4 * g + j
                    nc.tensor.matmul(
                        ps[j][:],
                        w1l_sb[32 * j:32 * j + KL, m * 128:(m + 1) * 128],
                        xl[32 * j:32 * j + KL, :],
                        start=False,
                        stop=True,
                        tile_position=(32 * j, 0),
                    )
                hts = []
                for j in range(4):
                    m = 4 * g + j
                    h = hpool.tile([128, WN], bf16, tag="h", name=f"h_{G}_{j}")
                    nc.scalar.activation(h[:], ps[j][:], Relu, bias=b1_sb[:, m:m + 1])
                    hts.append(h)
                hs[G] = hts

            def emit_l2_pack(G, half):
                """8 concurrent 32x32 tiles: block rows (2*half, 2*half+1) of
                the 4 m-tiles of group G, into 2 psum banks."""
                w, g = divmod(G, NGW)
                hts = hs[G]
                pk = [
                    ps2.tile([128, WN], f32, tag="p2", name=f"p2_{G}_{half}_{i}")
                    for i in range(2)
                ]
                for c in range(4):
                    for i in range(2):
                        r = 2 * half + i
                        nc.tensor.matmul(
                            pk[i][32 * c:32 * c + 32, :],
                            wb2_sb[32 * r:32 * r + 32,
                                   g * 128 + c * 32:g * 128 + c * 32 + 32],
                            hts[c][32 * r:32 * r + 32, :],
                            start=True,
                            stop=True,
                            tile_position=(32 * r, 32 * c),
                        )
                if half == 1:
                    hs.pop(G)
                for i in range(2):
                    band = 2 * half + i
                    h2 = h2pool.tile([128, WN], bf16, tag="h2",
                                     name=f"h2_{G}_{band}")
                    nc.vector.tensor_scalar(
                        h2[:], pk[i][:], bb_sb[:, 4 * g + band:4 * g + band + 1],
                        0.0, Add, Max,
                    )
                    h2s[(G, band)] = h2

            def emit_l3_quad(G):
                w, g = divmod(G, NGW)
                if g == 0:
                    pos[w] = ps3.tile([128, WN], f32, tag="po", name=f"po_{w}")
                po = pos[w]
                for band in range(NBAND):
                    t = 4 * g + band
                    nc.tensor.matmul(
                        po[32 * band:32 * band + OUT_DIM, :],
                        w3p_sb[:, t * OUT_DIM:(t + 1) * OUT_DIM],
                        h2s.pop((G, band))[:],
                        start=(g == 0),
                        stop=(g == NGW - 1),
                        skip_group_check=True,
                        tile_position=(0, 32 * band),
                    )
                if g == NGW - 1:
                    # band merge: only one PSUM operand per DVE op, so chain
                    # SBUF accumulators across the four bands (+ b3).
                    po = pos.pop(w)
                    t1 = opool.tile([OUT_DIM, WN], f32, tag="t1", name=f"t1_{w}")
                    nc.vector.tensor_scalar_add(t1[:], po[0:OUT_DIM, :], b3_sb[:])
                    t2 = opool.tile([OUT_DIM, WN], f32, tag="t2", name=f"t2_{w}")
                    nc.vector.tensor_tensor(
                        t2[:], t1[:], po[32:32 + OUT_DIM, :], Add
                    )
                    t3 = opool.tile([OUT_DIM, WN], f32, tag="t3", name=f"t3_{w}")
                    nc.vector.tensor_tensor(
                        t3[:], t2[:], po[64:64 + OUT_DIM, :], Add
                    )
                    ot = opool.tile([OUT_DIM, WN], f32, tag="ot", name=f"ot_{w}")
                    nc.vector.tensor_tensor(
                        ot[:], t3[:], po[96:96 + OUT_DIM, :], Add
                    )
                    nc.sync.dma_start(outT[:, w * WN:(w + 1) * WN], ot[:])

            for G in range(NGTOT + 2):
                if G < NGTOT:
                    emit_l1_fulls(G, (0, 1))
                if 1 <= G <= NGTOT:
                    emit_l2_pack(G - 1, 0)
                if G < NGTOT:
                    emit_l1_fulls(G, (2, 3))
                if 1 <= G <= NGTOT:
                    emit_l2_pack(G - 1, 1)
                if G < NGTOT:
                    emit_l1_quad(G)
                if G >= 2:
                    emit_l3_quad(G - 2)

    nc.compile()
    return nc


def _get_program(bc=BC):
    if bc not in _PROGRAM_CACHE:
        _PROGRAM_CACHE[bc] = _build_program(bc)
    return _PROGRAM_CACHE[bc]


def _prep_weights(W1, b1, Wb, bb, W3, b3):
    """Host-side packing of replicated weights into device layouts."""
    W1 = np.asarray(W1, dtype=np.float32)
    Wb = np.asarray(Wb, dtype=np.float32)
    W3 = np.asarray(W3, dtype=np.float32)
    bb = np.asarray(bb, dtype=np.float32)

    # W1T [128, NJ, K1, 512]: W1T[p, j, k, m] = W1.T[128k+p, 512j+m] -- the
    # per-partition-contiguous swizzle (3 KB DMA runs). W1L [128, 4096] holds
    # the 16 leftover feature rows replicated at partition offsets 0/32/64/96
    # for the row-group-packed leftover matmuls.
    NJ = NM // 4
    W1T = np.ascontiguousarray(
        W1.T[:K1 * 128].reshape(K1, 128, NJ, 512).transpose(1, 2, 0, 3)
    ).astype(BF16)
    W1L = np.zeros((128, HIDDEN), dtype=BF16)
    lo = W1.T[K1 * 128:IN_DIM].astype(BF16)
    for j in range(4):
        W1L[32 * j:32 * j + KL] = lo

    # Wb2 [128, NGW*128]: block (4*(4g+c)+r).T at [32r:+32, g*128+c*32:+32].
    Wb2 = np.zeros((128, NGW * 128), dtype=BF16)
    for g in range(NGW):
        for c in range(4):
            for r in range(4):
                blk = Wb[4 * (4 * g + c) + r].T.astype(BF16)  # [k, o]
                Wb2[32 * r:32 * r + 32,
                    g * 128 + c * 32:g * 128 + c * 32 + 32] = blk

    # Pack-output permutation: h2 band tile (g, band) partition 32c+o holds
    # hidden dim 128*(4g+c) + 32*band + o.
    # W3P [128, NM*10]: chunk t=4g+band at cols [t*10:+10]; W3P[p, t*10+o] =
    # W3[o, hid(g, band, p)].
    W3P = np.zeros((128, NM * OUT_DIM), dtype=BF16)
    bbP = np.zeros((128, 128), dtype=np.float32)
    for g in range(NGW):
        for band in range(4):
            t = 4 * g + band
            for c in range(4):
                hid0 = 128 * (4 * g + c) + 32 * band
                W3P[32 * c:32 * c + 32, t * OUT_DIM:(t + 1) * OUT_DIM] = (
                    W3[:, hid0:hid0 + 32].T.astype(BF16)
                )
                bbP[32 * c:32 * c + 32, t] = bb[hid0:hid0 + 32]

    NBC = NM + 128 + 1
    bcat = np.zeros((128, NBC), dtype=np.float32)
    bcat[:, 0:NM] = np.asarray(b1, np.float32).reshape(NM, 128).T
    bcat[:, NM:NM + 128] = bbP
    bcat[0:OUT_DIM, NM + 128] = np.asarray(b3, np.float32)
    return dict(W1T=W1T, W1L=W1L, Wb2=Wb2, W3P=W3P, bcat=bcat)


def _prep_x_shard(x, c, ncores=NCORES, bc=BC):
    xs = np.asarray(x[c * bc:(c + 1) * bc], dtype=np.float32).T.astype(BF16)  # [784, bc]
    nw = bc // WN
    # xT [128, nw, K1, WN]: xT[p, w, k, b] = x.T[128k+p, 512w+b]
    xT = np.ascontiguousarray(
        xs[:K1 * 128].reshape(K1, 128, nw, WN).transpose(1, 2, 0, 3)
    )
    xLs = np.zeros((128, bc), dtype=BF16)
    for j in range(4):
        xLs[32 * j:32 * j + KL] = xs[K1 * 128:IN_DIM]
    return xT, xLs


def run(x, W1, b1, Wb, bb, W3, b3, trace=False, tmpdir=None):
    """Run on 8 cores; returns (out [B,10] fp32, BassKernelResults)."""
    from concourse.bass_utils import run_bass_kernel_spmd

    nc = _get_program()
    wmap = _prep_weights(W1, b1, Wb, bb, W3, b3)
    in_maps = []
    for c in range(NCORES):
        m = dict(wmap)
        m["xT"], m["xL"] = _prep_x_shard(np.asarray(x), c)
        in_maps.append(m)

    res = run_bass_kernel_spmd(
        nc, in_maps, core_ids=list(range(NCORES)), trace=trace, tmpdir=tmpdir
    )
    out = np.concatenate(
        [np.asarray(r["outT"]).T for r in res.results], axis=0
    ).astype(np.float32)
    return out, res


def kernel(x, W1, b1, Wb, bb, W3, b3):
    out, _ = run(x, W1, b1, Wb, bb, W3, b3, trace=False)
    return out


# revision 14
# speedup vs baseline: 1.0031x; 1.0011x over previous
"""Trainium2 Bass kernel for BlockDiagMNIST MLP.

Reference computation (all fp32):
    h  = relu(x @ W1.T + b1)          x:[B,784], W1:[4096,784]    -> [B,4096]
    yb = blockdiag(h, Wb)             Wb:[128,32,32] (h2[b, 32n+o] = sum_k h[b,32n+k] Wb[n,o,k])
    h2 = relu(yb + bb)
    out = h2 @ W3.T + b3              W3:[10,4096]                -> [B,10]

Strategy: pure data-parallel over batch (B=32768 -> 4096 rows/core on 8 cores),
weights replicated.  All matmuls in bf16 (fp32 PSUM accumulation, fp32 biases).
On-chip layout is transposed ("hidden on partitions"): we compute
hT = W1 @ x.T per 512-column batch window.

Layer 2 (block-diagonal) runs as packs of eight concurrent 32x32 PE-array
tiles (tile_position row x col grid): tile (32r, 32c) applies one diagonal
block to h-slice [32r:32r+32] of m-tile (4g+c), writing psum partitions
[32c:32c+32].  Two packs (block rows 0,1 then 2,3) cover a 4-m-tile group
using 2 PSUM banks; outputs land hidden-permuted, which the host-side bb/W3
packing compensates.

Layer 3 (M=10) runs as four concurrent column-tiles: K-chunk t accumulates
into psum partitions [32*(t%4) : +10]; a final DVE pass sums the four bands
(cross-quadrant reads) and adds b3.

Host-side prep (free -- not on the device timeline): transpose + bf16-cast of
x and weights, block/bias/W3 permutation packing.
"""

import numpy as np
import ml_dtypes

B = 32768
IN_DIM = 784
HIDDEN = 4096
BLOCK = 32
NUM_BLOCKS = 128
OUT_DIM = 10
NCORES = 8
BC = B // NCORES          # batch rows per core (4096)
WN = 512                  # batch-window columns (one matmul free-dim)
K1 = 6                    # full 128-row K-chunks for layer 1 (features 0..767)
KL = 16                   # leftover K rows (features 768..783), row-group packed
NM = HIDDEN // 128        # 32 hidden tiles per window
NGW = NM // 4             # 8 groups of 4 m-tiles per window
NBAND = 4                 # layer-3 column-tile bands

BF16 = ml_dtypes.bfloat16

_PROGRAM_CACHE = {}


def _build_program(bc=BC):
    """Build (and bacc-compile) the per-core Bass program. bc = batch cols/core."""
    import concourse.mybir as mybir
    import concourse.tile as tile
    from concourse import bacc

    nw = bc // WN
    f32, bf16 = mybir.dt.float32, mybir.dt.bfloat16

    nc = bacc.Bacc("TRN2", target_bir_lowering=False, debug=False)

    # x / W1 stored pre-swizzled for per-partition contiguity (3 KB runs):
    # xT[p, w, k, b], W1T[p, j, k, m]
    xT = nc.dram_tensor("xT", [128, nw, K1, WN], bf16, kind="ExternalInput").ap()
    xL = nc.dram_tensor("xL", [128, bc], bf16, kind="ExternalInput").ap()
    w1t = nc.dram_tensor(
        "W1T", [128, NM, K1, 128], bf16, kind="ExternalInput"
    ).ap()
    w1l = nc.dram_tensor("W1L", [128, HIDDEN], bf16, kind="ExternalInput").ap()
    # Wb2: block (4*(4g+c)+r).T at [32r:32r+32, g*128+c*32 : +32]
    wb2 = nc.dram_tensor("Wb2", [128, NGW * 128], bf16, kind="ExternalInput").ap()
    # W3P: chunk t=4g+band -> [128, 10] at cols t*10; rows permuted to match the
    # pack layout (partition 32c+o of band tile <-> hidden 128*(4g+c)+32*band+o)
    w3p = nc.dram_tensor("W3P", [128, NM * OUT_DIM], bf16, kind="ExternalInput").ap()
    # biases packed into one tensor: cols 0..NM-1 = b1, NM..NM+127 = bbP
    # (col NM+4g+band, permuted like the L2 pack output), col NM+128 = b3
    NBC = NM + 128 + 1
    bcat = nc.dram_tensor("bcat", [128, NBC], f32, kind="ExternalInput").ap()
    outT = nc.dram_tensor("outT", [OUT_DIM, bc], f32, kind="ExternalOutput").ap()

    Relu = mybir.ActivationFunctionType.Relu
    Add = mybir.AluOpType.add
    Max = mybir.AluOpType.max

    MB = 4          # W1T column-block = MB m-tiles (DMA granularity for overlap)
    NJ = NM // MB   # 8 column blocks

    with tile.TileContext(nc) as tc:
        with (
            tc.tile_pool(name="const", bufs=1) as cpool,
            tc.tile_pool(name="xin", bufs=3) as xpool,
            tc.tile_pool(name="hbuf", bufs=10) as hpool,
            tc.tile_pool(name="h2buf", bufs=10) as h2pool,
            tc.tile_pool(name="obuf", bufs=4) as opool,
            tc.tile_pool(name="ps1", bufs=5, space="PSUM") as ps1,
            tc.tile_pool(name="ps2", bufs=2, space="PSUM") as ps2,
            tc.tile_pool(name="ps3", bufs=1, space="PSUM") as ps3,
        ):
            # HAM warmup on an un-DMA'd (garbage) SBUF tile: no data
            # dependency, so the PE clock gate starts ramping at t~0 and the
            # warmup stream covers the DMA-launch latency until real x/W1
            # data lands. Values are irrelevant (dummy psum, never read).
            gsb = cpool.tile([128, WN], bf16, name="warm_src")
            nc.vector.memset(gsb[:], 0)
            pw = ps2.tile([65, WN], f32, tag="p2", name="pwarm")
            for _ in range(20):
                nc.tensor.matmul(
                    pw[:, 0:65], gsb[:, 0:65], gsb[:, 0:65],
                    start=True, stop=True,
                )
            for _ in range(5):
                nc.tensor.matmul(
                    pw[:], gsb[:, 0:65], gsb[:],
                    start=True, stop=True,
                )

            def load_xt(w):
                """Per-window x tiles: two k-half DMAs (parallel queues, so the
                first window's data lands sooner) + the leftover rows."""
                KH = K1 // 2
                ta = xpool.tile([128, KH, WN], bf16, tag="xta", name=f"xta_{w}")
                nc.sync.dma_start(ta[:], xT[:, w, 0:KH, :])
                tb = xpool.tile([128, K1 - KH, WN], bf16, tag="xtb", name=f"xtb_{w}")
                nc.sync.dma_start(tb[:], xT[:, w, KH:K1, :])
                tl = xpool.tile([128, WN], bf16, tag="xl", name=f"xl_{w}")
                if w > 0:
                    nc.sync.dma_start(tl[:], xL[:, w * WN:(w + 1) * WN])
                return (ta, tb), tl

            # Window-0 x tile before everything else so PE starts early.
            xts = {0: load_xt(0)}

            # Small constants (ACT/DVE need them by the first relu).
            bc_sb = cpool.tile([128, NBC], f32)
            nc.sync.dma_start(bc_sb[:], bcat)
            b1_sb = bc_sb[:, 0:NM]
            bb_sb = bc_sb[:, NM:NM + 128]
            b3_sb = bc_sb[0:OUT_DIM, NM + 128:NBC]

            # W1T as NM per-m-tile DMAs [128, K1, 128] (196 KB each): the
            # startup-critical chain is just x window 0 + m-tile 0's weights,
            # so real matmuls start as soon as ~700 KB has landed.
            w1t_t = [None] * NM
            for m in range(NM):
                t = cpool.tile([128, K1, 128], bf16, name=f"w1m_{m}")
                nc.sync.dma_start(t[:], w1t[:, m, :, :])
                w1t_t[m] = t
                if m == 3:
                    # window-0 leftover rows: needed only at the end of the
                    # first group, so load after the critical first weights
                    nc.sync.dma_start(xts[0][1][:], xL[:, 0:WN])
                    w1l_sb = cpool.tile([128, HIDDEN], bf16)
                    nc.sync.dma_start(w1l_sb[:], w1l)
                    wb2_sb = cpool.tile([128, NGW * 128], bf16)
                    nc.sync.dma_start(wb2_sb[:], wb2)
                    w3p_sb = cpool.tile([128, NM * OUT_DIM], bf16)
                    nc.sync.dma_start(w3p_sb[:], w3p)

            # Software pipeline over G (global group index = window*NGW + g):
            #   L1(G) fulls interleaved with L2 packs of G-1 | L1 quad |
            #   L3 quad of G-2.
            NGTOT = nw * NGW
            pos = {}    # window -> psum accumulator for layer 3
            hs = {}     # G -> [4 h tiles]
            p1s_live = {}   # G -> [4 psum tiles] for the quad
            h2s = {}    # (G, band) -> h2 tile

            def emit_l1_fulls(G, js):
                w, g = divmod(G, NGW)
                if g == 0 and js[0] == 0 and w not in xts:
                    xts[w] = load_xt(w)
                xt, _ = xts[w]
                ps = p1s_live.setdefault(G, [None] * 4)
                KH = K1 // 2
                for j in js:
                    m = 4 * g + j
                    p1 = ps1.tile([128, WN], f32, tag="p1", name=f"p1_{G}_{j}")
                    for k in range(K1):
                        hi = k >= KH
                        nc.tensor.matmul(
                            p1[:],
                            w1t_t[m][:, k, :],
                            xt[hi][:, k - KH * hi, :],
                            start=(k == 0),
                            stop=False,
                        )
                    ps[j] = p1

            def emit_l1_quad(G):
                w, g = divmod(G, NGW)
                _, xl = xts[w]
                ps = p1s_live.pop(G)
                for j in range(4):
                    m = 4 * g + j
                    nc.tensor.matmul(
                        ps[j][:],
                        w1l_sb[32 * j:32 * j + KL, m * 128:(m + 1) * 128],
                        xl[32 * j:32 * j + KL, :],
                        start=False,
                        stop=True,
                        tile_position=(32 * j, 0),
                    )
                hts = []
                for j in range(4):
                    m = 4 * g + j
                    h = hpool.tile([128, WN], bf16, tag="h", name=f"h_{G}_{j}")
                    nc.scalar.activation(h[:], ps[j][:], Relu, bias=b1_sb[:, m:m + 1])
                    hts.append(h)
                hs[G] = hts

            def emit_l2_pack(G, half):
                """8 concurrent 32x32 tiles: block rows (2*half, 2*half+1) of
                the 4 m-tiles of group G, into 2 psum banks."""
                w, g = divmod(G, NGW)
                hts = hs[G]
                pk = [
                    ps2.tile([128, WN], f32, tag="p2", name=f"p2_{G}_{half}_{i}")
                    for i in range(2)
                ]
                for c in range(4):
                    for i in range(2):
                        r = 2 * half + i
                        nc.tensor.matmul(
                            pk[i][32 * c:32 * c + 32, :],
                            wb2_sb[32 * r:32 * r + 32,
                                   g * 128 + c * 32:g * 128 + c * 32 + 32],
                            hts[c][32 * r:32 * r + 32, :],
                            start=True,
                            stop=True,
                            tile_position=(32 * r, 32 * c),
                        )
                if half == 1:
                    hs.pop(G)
                for i in range(2):
                    band = 2 * half + i
                    h2 = h2pool.tile([128, WN], bf16, tag="h2",
                                     name=f"h2_{G}_{band}")
                    nc.vector.tensor_scalar(
                        h2[:], pk[i][:], bb_sb[:, 4 * g + band:4 * g + band + 1],
                        0.0, Add, Max,
                    )
                    h2s[(G, band)] = h2

            def emit_l3_quad(G):
                w, g = divmod(G, NGW)
                if g == 0:
                    pos[w] = ps3.tile([128, WN], f32, tag="po", name=f"po_{w}")
                po = pos[w]
                for band in range(NBAND):
                    t = 4 * g + band
                    nc.tensor.matmul(
                        po[32 * band:32 * band + OUT_DIM, :],
                        w3p_sb[:, t * OUT_DIM:(t + 1) * OUT_DIM],
                        h2s.pop((G, band))[:],
                        start=(g == 0),
                        stop=(g == NGW - 1),
                        skip_group_check=True,
                        tile_position=(0, 32 * band),
                    )
                if g == NGW - 1:
                    # band merge: only one PSUM operand per DVE op, so chain
                    # SBUF accumulators across the four bands (+ b3).
                    po = pos.pop(w)
                    t1 = opool.tile([OUT_DIM, WN], f32, tag="t1", name=f"t1_{w}")
                    nc.vector.tensor_scalar_add(t1[:], po[0:OUT_DIM, :], b3_sb[:])
                    t2 = opool.tile([OUT_DIM, WN], f32, tag="t2", name=f"t2_{w}")
                    nc.vector.tensor_tensor(
                        t2[:], t1[:], po[32:32 + OUT_DIM, :], Add
                    )
                    t3 = opool.tile([OUT_DIM, WN], f32, tag="t3", name=f"t3_{w}")
                    nc.vector.tensor_tensor(
                        t3[:], t2[:], po[64:64 + OUT_DIM, :], Add
                    )
                    ot = opool.tile([OUT_DIM, WN], f32, tag="ot", name=f"ot_{w}")
                    nc.vector.tensor_tensor(
                        ot[:], t3[:], po[96:96 + OUT_DIM, :], Add
                    )
                    nc.sync.dma_start(outT[:, w * WN:(w + 1) * WN], ot[:])

            for G in range(NGTOT + 2):
                if G < NGTOT:
                    emit_l1_fulls(G, (0, 1))
                if 1 <= G <= NGTOT:
                    emit_l2_pack(G - 1, 0)
                if G < NGTOT:
                    emit_l1_fulls(G, (2, 3))
                if 1 <= G <= NGTOT:
                    emit_l2_pack(G - 1, 1)
                if G < NGTOT:
                    emit_l1_quad(G)
                if G >= 2:
                    emit_l3_quad(G - 2)

    nc.compile()
    return nc


def _get_program(bc=BC):
    if bc not in _PROGRAM_CACHE:
        _PROGRAM_CACHE[bc] = _build_program(bc)
    return _PROGRAM_CACHE[bc]


def _prep_weights(W1, b1, Wb, bb, W3, b3):
    """Host-side packing of replicated weights into device layouts."""
    W1 = np.asarray(W1, dtype=np.float32)
    Wb = np.asarray(Wb, dtype=np.float32)
    W3 = np.asarray(W3, dtype=np.float32)
    bb = np.asarray(bb, dtype=np.float32)

    # W1T [128, NM, K1, 128]: W1T[p, m, k, c] = W1.T[128k+p, 128m+c] -- the
    # per-partition-contiguous swizzle (1.5 KB DMA runs, per-m-tile DMA
    # granularity). W1L [128, 4096] holds the 16 leftover feature rows
    # replicated at partition offsets 0/32/64/96 for the row-group-packed
    # leftover matmuls.
    W1T = np.ascontiguousarray(
        W1.T[:K1 * 128].reshape(K1, 128, NM, 128).transpose(1, 2, 0, 3)
    ).astype(BF16)
    W1L = np.zeros((128, HIDDEN), dtype=BF16)
    lo = W1.T[K1 * 128:IN_DIM].astype(BF16)
    for j in range(4):
        W1L[32 * j:32 * j + KL] = lo

    # Wb2 [128, NGW*128]: block (4*(4g+c)+r).T at [32r:+32, g*128+c*32:+32].
    Wb2 = np.zeros((128, NGW * 128), dtype=BF16)
    for g in range(NGW):
        for c in range(4):
            for r in range(4):
                blk = Wb[4 * (4 * g + c) + r].T.astype(BF16)  # [k, o]
                Wb2[32 * r:32 * r + 32,
                    g * 128 + c * 32:g * 128 + c * 32 + 32] = blk

    # Pack-output permutation: h2 band tile (g, band) partition 32c+o holds
    # hidden dim 128*(4g+c) + 32*band + o.
    # W3P [128, NM*10]: chunk t=4g+band at cols [t*10:+10]; W3P[p, t*10+o] =
    # W3[o, hid(g, band, p)].
    W3P = np.zeros((128, NM * OUT_DIM), dtype=BF16)
    bbP = np.zeros((128, 128), dtype=np.float32)
    for g in range(NGW):
        for band in range(4):
            t = 4 * g + band
            for c in range(4):
                hid0 = 128 * (4 * g + c) + 32 * band
                W3P[32 * c:32 * c + 32, t * OUT_DIM:(t + 1) * OUT_DIM] = (
                    W3[:, hid0:hid0 + 32].T.astype(BF16)
                )
                bbP[32 * c:32 * c + 32, t] = bb[hid0:hid0 + 32]

    NBC = NM + 128 + 1
    bcat = np.zeros((128, NBC), dtype=np.float32)
    bcat[:, 0:NM] = np.asarray(b1, np.float32).reshape(NM, 128).T
    bcat[:, NM:NM + 128] = bbP
    bcat[0:OUT_DIM, NM + 128] = np.asarray(b3, np.float32)
    return dict(W1T=W1T, W1L=W1L, Wb2=Wb2, W3P=W3P, bcat=bcat)


def _prep_x_shard(x, c, ncores=NCORES, bc=BC):
    xs = np.asarray(x[c * bc:(c + 1) * bc], dtype=np.float32).T.astype(BF16)  # [784, bc]
    nw = bc // WN
    # xT [128, nw, K1, WN]: xT[p, w, k, b] = x.T[128k+p, 512w+b]
    xT = np.ascontiguousarray(
        xs[:K1 * 128].reshape(K1, 128, nw, WN).transpose(1, 2, 0, 3)
    )
    xLs = np.zeros((128, bc), dtype=BF16)
    for j in range(4):
        xLs[32 * j:32 * j + KL] = xs[K1 * 128:IN_DIM]
    return xT, xLs


def run(x, W1, b1, Wb, bb, W3, b3, trace=False, tmpdir=None):
    """Run on 8 cores; returns (out [B,10] fp32, BassKernelResults)."""
    from concourse.bass_utils import run_bass_kernel_spmd

    nc = _get_program()
    wmap = _prep_weights(W1, b1, Wb, bb, W3, b3)
    in_maps = []
    for c in range(NCORES):
        m = dict(wmap)
        m["xT"], m["xL"] = _prep_x_shard(np.asarray(x), c)
        in_maps.append(m)

    res = run_bass_kernel_spmd(
        nc, in_maps, core_ids=list(range(NCORES)), trace=trace, tmpdir=tmpdir
    )
    out = np.concatenate(
        [np.asarray(r["outT"]).T for r in res.results], axis=0
    ).astype(np.float32)
    return out, res


def kernel(x, W1, b1, Wb, bb, W3, b3):
    out, _ = run(x, W1, b1, Wb, bb, W3, b3, trace=False)
    return out


# revision 15
# speedup vs baseline: 1.0035x; 1.0004x over previous
"""Trainium2 Bass kernel for BlockDiagMNIST MLP.

Reference computation (all fp32):
    h  = relu(x @ W1.T + b1)          x:[B,784], W1:[4096,784]    -> [B,4096]
    yb = blockdiag(h, Wb)             Wb:[128,32,32] (h2[b, 32n+o] = sum_k h[b,32n+k] Wb[n,o,k])
    h2 = relu(yb + bb)
    out = h2 @ W3.T + b3              W3:[10,4096]                -> [B,10]

Strategy: pure data-parallel over batch (B=32768 -> 4096 rows/core on 8 cores),
weights replicated.  All matmuls in bf16 (fp32 PSUM accumulation, fp32 biases).
On-chip layout is transposed ("hidden on partitions"): we compute
hT = W1 @ x.T per 512-column batch window.

Layer 2 (block-diagonal) runs as packs of eight concurrent 32x32 PE-array
tiles (tile_position row x col grid): tile (32r, 32c) applies one diagonal
block to h-slice [32r:32r+32] of m-tile (4g+c), writing psum partitions
[32c:32c+32].  Two packs (block rows 0,1 then 2,3) cover a 4-m-tile group
using 2 PSUM banks; outputs land hidden-permuted, which the host-side bb/W3
packing compensates.

Layer 3 (M=10) runs as four concurrent column-tiles: K-chunk t accumulates
into psum partitions [32*(t%4) : +10]; a final DVE pass sums the four bands
(cross-quadrant reads) and adds b3.

Host-side prep (free -- not on the device timeline): transpose + bf16-cast of
x and weights, block/bias/W3 permutation packing.
"""

import numpy as np
import ml_dtypes

B = 32768
IN_DIM = 784
HIDDEN = 4096
BLOCK = 32
NUM_BLOCKS = 128
OUT_DIM = 10
NCORES = 8
BC = B // NCORES          # batch rows per core (4096)
WN = 512                  # batch-window columns (one matmul free-dim)
K1 = 6                    # full 128-row K-chunks for layer 1 (features 0..767)
KL = 16                   # leftover K rows (features 768..783), row-group packed
NM = HIDDEN // 128        # 32 hidden tiles per window
NGW = NM // 4             # 8 groups of 4 m-tiles per window
NBAND = 4                 # layer-3 column-tile bands

BF16 = ml_dtypes.bfloat16

_PROGRAM_CACHE = {}


def _build_program(bc=BC):
    """Build (and bacc-compile) the per-core Bass program. bc = batch cols/core."""
    import concourse.mybir as mybir
    import concourse.tile as tile
    from concourse import bacc

    nw = bc // WN
    f32, bf16 = mybir.dt.float32, mybir.dt.bfloat16

    nc = bacc.Bacc("TRN2", target_bir_lowering=False, debug=False)

    # x / W1 stored pre-swizzled for per-partition contiguity (3 KB runs):
    # xT[p, w, k, b], W1T[p, j, k, m]
    xT = nc.dram_tensor("xT", [128, nw, K1, WN], bf16, kind="ExternalInput").ap()
    xL = nc.dram_tensor("xL", [128, bc], bf16, kind="ExternalInput").ap()
    w1t = nc.dram_tensor(
        "W1T", [128, NM, K1, 128], bf16, kind="ExternalInput"
    ).ap()
    w1l = nc.dram_tensor("W1L", [128, HIDDEN], bf16, kind="ExternalInput").ap()
    # Wb2: block (4*(4g+c)+r).T at [32r:32r+32, g*128+c*32 : +32]
    wb2 = nc.dram_tensor("Wb2", [128, NGW * 128], bf16, kind="ExternalInput").ap()
    # W3P: chunk t=4g+band -> [128, 10] at cols t*10; rows permuted to match the
    # pack layout (partition 32c+o of band tile <-> hidden 128*(4g+c)+32*band+o)
    w3p = nc.dram_tensor("W3P", [128, NM * OUT_DIM], bf16, kind="ExternalInput").ap()
    # biases packed into one tensor: cols 0..NM-1 = b1, NM..NM+127 = bbP
    # (col NM+4g+band, permuted like the L2 pack output), col NM+128 = b3
    NBC = NM + 128 + 1
    bcat = nc.dram_tensor("bcat", [128, NBC], f32, kind="ExternalInput").ap()
    outT = nc.dram_tensor("outT", [OUT_DIM, bc], f32, kind="ExternalOutput").ap()

    Relu = mybir.ActivationFunctionType.Relu
    Add = mybir.AluOpType.add
    Max = mybir.AluOpType.max

    MB = 4          # W1T column-block = MB m-tiles (DMA granularity for overlap)
    NJ = NM // MB   # 8 column blocks

    with tile.TileContext(nc) as tc:
        with (
            tc.tile_pool(name="const", bufs=1) as cpool,
            tc.tile_pool(name="xin", bufs=3) as xpool,
            tc.tile_pool(name="hbuf", bufs=10) as hpool,
            tc.tile_pool(name="h2buf", bufs=10) as h2pool,
            tc.tile_pool(name="obuf", bufs=4) as opool,
            tc.tile_pool(name="ps1", bufs=5, space="PSUM") as ps1,
            tc.tile_pool(name="ps2", bufs=2, space="PSUM") as ps2,
            tc.tile_pool(name="ps3", bufs=1, space="PSUM") as ps3,
        ):
            # HAM warmup on an un-DMA'd (garbage) SBUF tile: no data
            # dependency, so the PE clock gate starts ramping at t~0 and the
            # warmup stream covers the DMA-launch latency until real x/W1
            # data lands. Values are irrelevant (dummy psum, never read).
            gsb = cpool.tile([128, WN], bf16, name="warm_src")
            nc.vector.memset(gsb[:], 0)
            pw = ps2.tile([65, WN], f32, tag="p2", name="pwarm")
            for _ in range(20):
                nc.tensor.matmul(
                    pw[:, 0:65], gsb[:, 0:65], gsb[:, 0:65],
                    start=True, stop=True,
                )
            for _ in range(5):
                nc.tensor.matmul(
                    pw[:], gsb[:, 0:65], gsb[:],
                    start=True, stop=True,
                )

            def load_xt(w):
                """Per-window x tiles: two k-half DMAs (parallel queues, so the
                first window's data lands sooner) + the leftover rows."""
                KH = K1 // 2
                ta = xpool.tile([128, KH, WN], bf16, tag="xta", name=f"xta_{w}")
                nc.sync.dma_start(ta[:], xT[:, w, 0:KH, :])
                tb = xpool.tile([128, K1 - KH, WN], bf16, tag="xtb", name=f"xtb_{w}")
                nc.sync.dma_start(tb[:], xT[:, w, KH:K1, :])
                tl = xpool.tile([128, WN], bf16, tag="xl", name=f"xl_{w}")
                if w > 0:
                    nc.sync.dma_start(tl[:], xL[:, w * WN:(w + 1) * WN])
                return (ta, tb), tl

            # Window-0 x tile before everything else so PE starts early.
            xts = {0: load_xt(0)}

            # Small constants (ACT/DVE need them by the first relu).
            bc_sb = cpool.tile([128, NBC], f32)
            nc.sync.dma_start(bc_sb[:], bcat)
            b1_sb = bc_sb[:, 0:NM]
            bb_sb = bc_sb[:, NM:NM + 128]
            b3_sb = bc_sb[0:OUT_DIM, NM + 128:NBC]

            # W1T as NM per-m-tile DMAs [128, K1, 128] (196 KB each): the
            # startup-critical chain is just x window 0 + m-tile 0's weights,
            # so real matmuls start as soon as ~700 KB has landed. The
            # leftover/L2/L3 weights are split per group and interleaved so
            # each group's full weight set arrives just ahead of its compute.
            w1l_sb = cpool.tile([128, HIDDEN], bf16)
            wb2_sb = cpool.tile([128, NGW * 128], bf16)
            w3p_sb = cpool.tile([128, NM * OUT_DIM], bf16)
            w1t_t = [None] * NM
            for m in range(NM):
                t = cpool.tile([128, K1, 128], bf16, name=f"w1m_{m}")
                nc.sync.dma_start(t[:], w1t[:, m, :, :])
                w1t_t[m] = t
                if m == 3:
                    # window-0 leftover rows (needed at group 0's quad)
                    nc.sync.dma_start(xts[0][1][:], xL[:, 0:WN])
                if m % 4 == 3:
                    g = m // 4
                    nc.sync.dma_start(
                        w1l_sb[:, g * WN:(g + 1) * WN],
                        w1l[:, g * WN:(g + 1) * WN],
                    )
                    nc.sync.dma_start(
                        wb2_sb[:, g * 128:(g + 1) * 128],
                        wb2[:, g * 128:(g + 1) * 128],
                    )
                    nc.sync.dma_start(
                        w3p_sb[:, g * 4 * OUT_DIM:(g + 1) * 4 * OUT_DIM],
                        w3p[:, g * 4 * OUT_DIM:(g + 1) * 4 * OUT_DIM],
                    )

            # Software pipeline over G (global group index = window*NGW + g):
            #   L1(G) fulls interleaved with L2 packs of G-1 | L1 quad |
            #   L3 quad of G-2.
            NGTOT = nw * NGW
            pos = {}    # window -> psum accumulator for layer 3
            hs = {}     # G -> [4 h tiles]
            p1s_live = {}   # G -> [4 psum tiles] for the quad
            h2s = {}    # (G, band) -> h2 tile

            def emit_l1_fulls(G, js):
                w, g = divmod(G, NGW)
                if g == 0 and js[0] == 0 and w not in xts:
                    xts[w] = load_xt(w)
                xt, _ = xts[w]
                ps = p1s_live.setdefault(G, [None] * 4)
                KH = K1 // 2
                for j in js:
                    m = 4 * g + j
                    p1 = ps1.tile([128, WN], f32, tag="p1", name=f"p1_{G}_{j}")
                    for k in range(K1):
                        hi = k >= KH
                        nc.tensor.matmul(
                            p1[:],
                            w1t_t[m][:, k, :],
                            xt[hi][:, k - KH * hi, :],
                            start=(k == 0),
                            stop=False,
                        )
                    ps[j] = p1

            def emit_l1_quad(G):
                w, g = divmod(G, NGW)
                _, xl = xts[w]
                ps = p1s_live.pop(G)
                for j in range(4):
                    m = 4 * g + j
                    nc.tensor.matmul(
                        ps[j][:],
                        w1l_sb[32 * j:32 * j + KL, m * 128:(m + 1) * 128],
                        xl[32 * j:32 * j + KL, :],
                        start=False,
                        stop=True,
                        tile_position=(32 * j, 0),
                    )
                hts = []
                for j in range(4):
                    m = 4 * g + j
                    h = hpool.tile([128, WN], bf16, tag="h", name=f"h_{G}_{j}")
                    nc.scalar.activation(h[:], ps[j][:], Relu, bias=b1_sb[:, m:m + 1])
                    hts.append(h)
                hs[G] = hts

            def emit_l2_pack(G, half):
                """8 concurrent 32x32 tiles: block rows (2*half, 2*half+1) of
                the 4 m-tiles of group G, into 2 psum banks."""
                w, g = divmod(G, NGW)
                hts = hs[G]
                pk = [
                    ps2.tile([128, WN], f32, tag="p2", name=f"p2_{G}_{half}_{i}")
                    for i in range(2)
                ]
                for c in range(4):
                    for i in range(2):
                        r = 2 * half + i
                        nc.tensor.matmul(
                            pk[i][32 * c:32 * c + 32, :],
                            wb2_sb[32 * r:32 * r + 32,
                                   g * 128 + c * 32:g * 128 + c * 32 + 32],
                            hts[c][32 * r:32 * r + 32, :],
                            start=True,
                            stop=True,
                            tile_position=(32 * r, 32 * c),
                        )
                if half == 1:
                    hs.pop(G)
                for i in range(2):
                    band = 2 * half + i
                    h2 = h2pool.tile([128, WN], bf16, tag="h2",
                                     name=f"h2_{G}_{band}")
                    nc.vector.tensor_scalar(
                        h2[:], pk[i][:], bb_sb[:, 4 * g + band:4 * g + band + 1],
                        0.0, Add, Max,
                    )
                    h2s[(G, band)] = h2

            def emit_l3_quad(G):
                w, g = divmod(G, NGW)
                if g == 0:
                    pos[w] = ps3.tile([128, WN], f32, tag="po", name=f"po_{w}")
                po = pos[w]
                for band in range(NBAND):
                    t = 4 * g + band
                    nc.tensor.matmul(
                        po[32 * band:32 * band + OUT_DIM, :],
                        w3p_sb[:, t * OUT_DIM:(t + 1) * OUT_DIM],
                        h2s.pop((G, band))[:],
                        start=(g == 0),
                        stop=(g == NGW - 1),
                        skip_group_check=True,
                        tile_position=(0, 32 * band),
                    )
                if g == NGW - 1:
                    # band merge: only one PSUM operand per DVE op, so chain
                    # SBUF accumulators across the four bands (+ b3).
                    po = pos.pop(w)
                    t1 = opool.tile([OUT_DIM, WN], f32, tag="t1", name=f"t1_{w}")
                    nc.vector.tensor_scalar_add(t1[:], po[0:OUT_DIM, :], b3_sb[:])
                    t2 = opool.tile([OUT_DIM, WN], f32, tag="t2", name=f"t2_{w}")
                    nc.vector.tensor_tensor(
                        t2[:], t1[:], po[32:32 + OUT_DIM, :], Add
                    )
                    t3 = opool.tile([OUT_DIM, WN], f32, tag="t3", name=f"t3_{w}")
                    nc.vector.tensor_tensor(
                        t3[:], t2[:], po[64:64 + OUT_DIM, :], Add
                    )
                    ot = opool.tile([OUT_DIM, WN], f32, tag="ot", name=f"ot_{w}")
                    nc.vector.tensor_tensor(
                        ot[:], t3[:], po[96:96 + OUT_DIM, :], Add
                    )
                    nc.sync.dma_start(outT[:, w * WN:(w + 1) * WN], ot[:])

            for G in range(NGTOT + 2):
                if G < NGTOT:
                    emit_l1_fulls(G, (0, 1))
                if 1 <= G <= NGTOT:
                    emit_l2_pack(G - 1, 0)
                if G < NGTOT:
                    emit_l1_fulls(G, (2, 3))
                if 1 <= G <= NGTOT:
                    emit_l2_pack(G - 1, 1)
                if G < NGTOT:
                    emit_l1_quad(G)
                if G >= 2:
                    emit_l3_quad(G - 2)

    nc.compile()
    return nc


def _get_program(bc=BC):
    if bc not in _PROGRAM_CACHE:
        _PROGRAM_CACHE[bc] = _build_program(bc)
    return _PROGRAM_CACHE[bc]


def _prep_weights(W1, b1, Wb, bb, W3, b3):
    """Host-side packing of replicated weights into device layouts."""
    W1 = np.asarray(W1, dtype=np.float32)
    Wb = np.asarray(Wb, dtype=np.float32)
    W3 = np.asarray(W3, dtype=np.float32)
    bb = np.asarray(bb, dtype=np.float32)

    # W1T [128, NM, K1, 128]: W1T[p, m, k, c] = W1.T[128k+p, 128m+c] -- the
    # per-partition-contiguous swizzle (1.5 KB DMA runs, per-m-tile DMA
    # granularity). W1L [128, 4096] holds the 16 leftover feature rows
    # replicated at partition offsets 0/32/64/96 for the row-group-packed
    # leftover matmuls.
    W1T = np.ascontiguousarray(
        W1.T[:K1 * 128].reshape(K1, 128, NM, 128).transpose(1, 2, 0, 3)
    ).astype(BF16)
    W1L = np.zeros((128, HIDDEN), dtype=BF16)
    lo = W1.T[K1 * 128:IN_DIM].astype(BF16)
    for j in range(4):
        W1L[32 * j:32 * j + KL] = lo

    # Wb2 [128, NGW*128]: block (4*(4g+c)+r).T at [32r:+32, g*128+c*32:+32].
    Wb2 = np.zeros((128, NGW * 128), dtype=BF16)
    for g in range(NGW):
        for c in range(4):
            for r in range(4):
                blk = Wb[4 * (4 * g + c) + r].T.astype(BF16)  # [k, o]
                Wb2[32 * r:32 * r + 32,
                    g * 128 + c * 32:g * 128 + c * 32 + 32] = blk

    # Pack-output permutation: h2 band tile (g, band) partition 32c+o holds
    # hidden dim 128*(4g+c) + 32*band + o.
    # W3P [128, NM*10]: chunk t=4g+band at cols [t*10:+10]; W3P[p, t*10+o] =
    # W3[o, hid(g, band, p)].
    W3P = np.zeros((128, NM * OUT_DIM), dtype=BF16)
    bbP = np.zeros((128, 128), dtype=np.float32)
    for g in range(NGW):
        for band in range(4):
            t = 4 * g + band
            for c in range(4):
                hid0 = 128 * (4 * g + c) + 32 * band
                W3P[32 * c:32 * c + 32, t * OUT_DIM:(t + 1) * OUT_DIM] = (
                    W3[:, hid0:hid0 + 32].T.astype(BF16)
                )
                bbP[32 * c:32 * c + 32, t] = bb[hid0:hid0 + 32]

    NBC = NM + 128 + 1
    bcat = np.zeros((128, NBC), dtype=np.float32)
    bcat[:, 0:NM] = np.asarray(b1, np.float32).reshape(NM, 128).T
    bcat[:, NM:NM + 128] = bbP
    bcat[0:OUT_DIM, NM + 128] = np.asarray(b3, np.float32)
    return dict(W1T=W1T, W1L=W1L, Wb2=Wb2, W3P=W3P, bcat=bcat)


def _prep_x_shard(x, c, ncores=NCORES, bc=BC):
    xs = np.asarray(x[c * bc:(c + 1) * bc], dtype=np.float32).T.astype(BF16)  # [784, bc]
    nw = bc // WN
    # xT [128, nw, K1, WN]: xT[p, w, k, b] = x.T[128k+p, 512w+b]
    xT = np.ascontiguousarray(
        xs[:K1 * 128].reshape(K1, 128, nw, WN).transpose(1, 2, 0, 3)
    )
    xLs = np.zeros((128, bc), dtype=BF16)
    for j in range(4):
        xLs[32 * j:32 * j + KL] = xs[K1 * 128:IN_DIM]
    return xT, xLs


def run(x, W1, b1, Wb, bb, W3, b3, trace=False, tmpdir=None):
    """Run on 8 cores; returns (out [B,10] fp32, BassKernelResults)."""
    from concourse.bass_utils import run_bass_kernel_spmd

    nc = _get_program()
    wmap = _prep_weights(W1, b1, Wb, bb, W3, b3)
    in_maps = []
    for c in range(NCORES):
        m = dict(wmap)
        m["xT"], m["xL"] = _prep_x_shard(np.asarray(x), c)
        in_maps.append(m)

    res = run_bass_kernel_spmd(
        nc, in_maps, core_ids=list(range(NCORES)), trace=trace, tmpdir=tmpdir
    )
    out = np.concatenate(
        [np.asarray(r["outT"]).T for r in res.results], axis=0
    ).astype(np.float32)
    return out, res


def kernel(x, W1, b1, Wb, bb, W3, b3):
    out, _ = run(x, W1, b1, Wb, bb, W3, b3, trace=False)
    return out


# revision 26
# speedup vs baseline: 1.0486x; 1.0450x over previous
"""Trainium2 Bass kernel for BlockDiagMNIST MLP.

Reference computation (all fp32):
    h  = relu(x @ W1.T + b1)          x:[B,784], W1:[4096,784]    -> [B,4096]
    yb = blockdiag(h, Wb)             Wb:[128,32,32] (h2[b, 32n+o] = sum_k h[b,32n+k] Wb[n,o,k])
    h2 = relu(yb + bb)
    out = h2 @ W3.T + b3              W3:[10,4096]                -> [B,10]

Strategy: pure data-parallel over batch (B=32768 -> 4096 rows/core on 8 cores),
weights replicated.  All matmuls in bf16 (fp32 PSUM accumulation, fp32 biases).
On-chip layout is transposed ("hidden on partitions"): we compute
hT = W1 @ x.T per 512-column batch window.

Layer 2 (block-diagonal) runs as packs of eight concurrent 32x32 PE-array
tiles (tile_position row x col grid): tile (32r, 32c) applies one diagonal
block to h-slice [32r:32r+32] of m-tile (4g+c), writing psum partitions
[32c:32c+32].  Two packs (block rows 0,1 then 2,3) cover a 4-m-tile group
using 2 PSUM banks; outputs land hidden-permuted, which the host-side bb/W3
packing compensates.

Layer 3 (M=10) runs as four concurrent column-tiles: K-chunk t accumulates
into psum partitions [32*(t%4) : +10]; a final DVE pass sums the four bands
(cross-quadrant reads) and adds b3.

Host-side prep (free -- not on the device timeline): transpose + bf16-cast of
x and weights, block/bias/W3 permutation packing.
"""

import numpy as np
import ml_dtypes

B = 32768
IN_DIM = 784
HIDDEN = 4096
BLOCK = 32
NUM_BLOCKS = 128
OUT_DIM = 10
NCORES = 8
BC = B // NCORES          # batch rows per core (4096)
WN = 512                  # batch-window columns (one matmul free-dim)
K1 = 6                    # full 128-row K-chunks for layer 1 (features 0..767)
KL = 16                   # leftover K rows (features 768..783), row-group packed
NM = HIDDEN // 128        # 32 hidden tiles per window
NGW = NM // 4             # 8 groups of 4 m-tiles per window
NBAND = 4                 # layer-3 column-tile bands
NM8 = 20                  # m-tiles whose first 2 K-chunks run in fp8 DoubleRow
SW8 = 32.0                # fp8 weight scale (x scaled by 1/SW8: product scale 1)

BF16 = ml_dtypes.bfloat16
E4M3 = ml_dtypes.float8_e4m3   # TRN FP8_EXP4-compatible (max +-240)

_PROGRAM_CACHE = {}


def _build_program(bc=BC):
    """Build (and bacc-compile) the per-core Bass program. bc = batch cols/core."""
    import concourse.mybir as mybir
    import concourse.tile as tile
    from concourse import bacc

    nw = bc // WN
    f32, bf16 = mybir.dt.float32, mybir.dt.bfloat16
    fp8 = mybir.dt.float8e4

    nc = bacc.Bacc("TRN2", target_bir_lowering=False, debug=False)

    # x / W1 stored pre-swizzled for per-partition contiguity (3 KB runs):
    # xT[p, w, k, b], W1T[p, j, k, m]
    xT = nc.dram_tensor("xT", [128, nw, K1, WN], bf16, kind="ExternalInput").ap()
    x8d = nc.dram_tensor("X8", [128, nw, 2, WN], fp8, kind="ExternalInput").ap()
    xL = nc.dram_tensor("xL", [128, bc], bf16, kind="ExternalInput").ap()
    w1t = nc.dram_tensor(
        "W1T", [128, NM, K1, 128], bf16, kind="ExternalInput"
    ).ap()
    w18d = nc.dram_tensor(
        "W18", [128, NM8, 2, 128], fp8, kind="ExternalInput"
    ).ap()
    w1l = nc.dram_tensor("W1L", [128, HIDDEN], bf16, kind="ExternalInput").ap()
    # Wb2: block (4*(4g+c)+r).T at [32r:32r+32, g*128+c*32 : +32]
    wb2 = nc.dram_tensor("Wb2", [128, NGW * 128], bf16, kind="ExternalInput").ap()
    # W3P: chunk t=4g+band -> [128, 10] at cols t*10; rows permuted to match the
    # pack layout (partition 32c+o of band tile <-> hidden 128*(4g+c)+32*band+o)
    w3p = nc.dram_tensor("W3P", [128, NM * OUT_DIM], bf16, kind="ExternalInput").ap()
    # biases packed into one tensor: cols 0..NM-1 = b1, NM..NM+127 = bbP
    # (col NM+4g+band, permuted like the L2 pack output), col NM+128 = b3
    NBC = NM + 128 + 1
    bcat = nc.dram_tensor("bcat", [128, NBC], f32, kind="ExternalInput").ap()
    outT = nc.dram_tensor("outT", [OUT_DIM, bc], f32, kind="ExternalOutput").ap()

    Relu = mybir.ActivationFunctionType.Relu
    Add = mybir.AluOpType.add
    Max = mybir.AluOpType.max

    MB = 4          # W1T column-block = MB m-tiles (DMA granularity for overlap)
    NJ = NM // MB   # 8 column blocks

    with tile.TileContext(nc) as tc:
        with (
            tc.tile_pool(name="const", bufs=1) as cpool,
            tc.tile_pool(name="xin", bufs=3) as xpool,
            tc.tile_pool(name="hbuf", bufs=10) as hpool,
            tc.tile_pool(name="h2buf", bufs=10) as h2pool,
            tc.tile_pool(name="obuf", bufs=4) as opool,
            tc.tile_pool(name="ps1", bufs=5, space="PSUM") as ps1,
            tc.tile_pool(name="ps2", bufs=2, space="PSUM") as ps2,
            tc.tile_pool(name="ps3", bufs=1, space="PSUM") as ps3,
        ):
            # HAM warmup on an un-DMA'd (garbage) SBUF tile: no data
            # dependency, so the PE clock gate starts ramping at t~0 and the
            # warmup stream covers the DMA-launch latency until real x/W1
            # data lands. Values are irrelevant (dummy psum, never read).
            gsb = cpool.tile([128, WN], bf16, name="warm_src")
            nc.vector.memset(gsb[:], 0)
            pw = ps2.tile([65, WN], f32, tag="p2", name="pwarm")
            for _ in range(20):
                nc.tensor.matmul(
                    pw[:, 0:65], gsb[:, 0:65], gsb[:, 0:65],
                    start=True, stop=True,
                )
            for _ in range(5):
                nc.tensor.matmul(
                    pw[:], gsb[:, 0:65], gsb[:],
                    start=True, stop=True,
                )

            def load_xt(w):
                """Per-window x tiles: two k-half DMAs (parallel queues, so the
                first window's data lands sooner) + the leftover rows."""
                KH = K1 // 2
                t8 = xpool.tile([128, 2, WN], fp8, tag="x8", name=f"x8_{w}")
                nc.sync.dma_start(t8[:], x8d[:, w, :, :])
                ta = xpool.tile([128, KH, WN], bf16, tag="xta", name=f"xta_{w}")
                nc.sync.dma_start(ta[:], xT[:, w, 0:KH, :])
                tb = xpool.tile([128, K1 - KH, WN], bf16, tag="xtb", name=f"xtb_{w}")
                nc.sync.dma_start(tb[:], xT[:, w, KH:K1, :])
                tl = xpool.tile([128, WN], bf16, tag="xl", name=f"xl_{w}")
                if w > 0:
                    nc.sync.dma_start(tl[:], xL[:, w * WN:(w + 1) * WN])
                return (ta, tb, t8), tl

            # Window-0 x tile before everything else so PE starts early.
            xts = {0: load_xt(0)}

            # Small constants (ACT/DVE need them by the first relu).
            bc_sb = cpool.tile([128, NBC], f32)
            nc.sync.dma_start(bc_sb[:], bcat)
            b1_sb = bc_sb[:, 0:NM]
            bb_sb = bc_sb[:, NM:NM + 128]
            b3_sb = bc_sb[0:OUT_DIM, NM + 128:NBC]

            # W1T as NM per-m-tile DMAs [128, K1, 128] (196 KB each): the
            # startup-critical chain is just x window 0 + m-tile 0's weights,
            # so real matmuls start as soon as ~700 KB has landed. The
            # leftover/L2/L3 weights are split per group and interleaved so
            # each group's full weight set arrives just ahead of its compute.
            w1l_sb = cpool.tile([128, HIDDEN], bf16)
            wb2_sb = cpool.tile([128, NGW * 128], bf16)
            w3p_sb = cpool.tile([128, NM * OUT_DIM], bf16)
            w1t_t = [None] * NM
            w18_t = [None] * NM8
            for m in range(NM):
                if m < NM8:
                    # fp8 m-tile: chunks 0-1 come from the fp8 pair tensor,
                    # bf16 weights only for chunks 2..5
                    t8 = cpool.tile([128, 2, 128], fp8, name=f"w18m_{m}")
                    nc.sync.dma_start(t8[:], w18d[:, m, :, :])
                    w18_t[m] = t8
                    t = cpool.tile([128, K1 - 2, 128], bf16, name=f"w1m_{m}")
                    nc.sync.dma_start(t[:], w1t[:, m, 2:K1, :])
                else:
                    t = cpool.tile([128, K1, 128], bf16, name=f"w1m_{m}")
                    nc.sync.dma_start(t[:], w1t[:, m, :, :])
                w1t_t[m] = t
                if m == 3:
                    # window-0 leftover rows (needed at group 0's quad)
                    nc.sync.dma_start(xts[0][1][:], xL[:, 0:WN])
                if m % 4 == 3:
                    g = m // 4
                    nc.sync.dma_start(
                        w1l_sb[:, g * WN:(g + 1) * WN],
                        w1l[:, g * WN:(g + 1) * WN],
                    )
                    nc.sync.dma_start(
                        wb2_sb[:, g * 128:(g + 1) * 128],
                        wb2[:, g * 128:(g + 1) * 128],
                    )
                    nc.sync.dma_start(
                        w3p_sb[:, g * 4 * OUT_DIM:(g + 1) * 4 * OUT_DIM],
                        w3p[:, g * 4 * OUT_DIM:(g + 1) * 4 * OUT_DIM],
                    )

            # Software pipeline over G (global group index = window*NGW + g):
            #   L1(G) fulls interleaved with L2 packs of G-1 | L1 quad |
            #   L3 quad of G-2.
            NGTOT = nw * NGW
            pos = {}    # window -> psum accumulator for layer 3
            hs = {}     # G -> [4 h tiles]
            p1s_live = {}   # G -> [4 psum tiles] for the quad
            h2s = {}    # (G, band) -> h2 tile

            DR = mybir.MatmulPerfMode.DoubleRow

            def emit_l1_fulls(G, js):
                w, g = divmod(G, NGW)
                if g == 0 and js[0] == 0 and w not in xts:
                    xts[w] = load_xt(w)
                xt, _ = xts[w]
                ps = p1s_live.setdefault(G, [None] * 4)
                KH = K1 // 2
                for j in js:
                    m = 4 * g + j
                    p1 = ps1.tile([128, WN], f32, tag="p1", name=f"p1_{G}_{j}")
                    if m < NM8:
                        # chunks 0-1 as one fp8 DoubleRow matmul
                        nc.tensor.matmul(
                            p1[:],
                            w18_t[m][:],
                            xt[2][:],
                            start=True,
                            stop=False,
                            perf_mode=DR,
                        )
                        krange = range(2, K1)
                        woff = 2
                    else:
                        krange = range(K1)
                        woff = 0
                    for k in krange:
                        hi = k >= KH
                        nc.tensor.matmul(
                            p1[:],
                            w1t_t[m][:, k - woff, :],
                            xt[hi][:, k - KH * hi, :],
                            start=(k == 0 and woff == 0),
                            stop=False,
                        )
                    ps[j] = p1

            def emit_l1_quad(G):
                w, g = divmod(G, NGW)
                _, xl = xts[w]
                ps = p1s_live.pop(G)
                for j in range(4):
                    m = 4 * g + j
                    nc.tensor.matmul(
                        ps[j][:],
                        w1l_sb[32 * j:32 * j + KL, m * 128:(m + 1) * 128],
                        xl[32 * j:32 * j + KL, :],
                        start=False,
                        stop=True,
                        tile_position=(32 * j, 0),
                    )
                hts = []
                for j in range(4):
                    m = 4 * g + j
                    h = hpool.tile([128, WN], bf16, tag="h", name=f"h_{G}_{j}")
                    nc.scalar.activation(h[:], ps[j][:], Relu, bias=b1_sb[:, m:m + 1])
                    hts.append(h)
                hs[G] = hts

            def emit_l2_pack(G, half):
                """8 concurrent 32x32 tiles: block rows (2*half, 2*half+1) of
                the 4 m-tiles of group G, into 2 psum banks."""
                w, g = divmod(G, NGW)
                hts = hs[G]
                pk = [
                    ps2.tile([128, WN], f32, tag="p2", name=f"p2_{G}_{half}_{i}")
                    for i in range(2)
                ]
                for c in range(4):
                    for i in range(2):
                        r = 2 * half + i
                        nc.tensor.matmul(
                            pk[i][32 * c:32 * c + 32, :],
                            wb2_sb[32 * r:32 * r + 32,
                                   g * 128 + c * 32:g * 128 + c * 32 + 32],
                            hts[c][32 * r:32 * r + 32, :],
                            start=True,
                            stop=True,
                            tile_position=(32 * r, 32 * c),
                        )
                if half == 1:
                    hs.pop(G)
                for i in range(2):
                    band = 2 * half + i
                    h2 = h2pool.tile([128, WN], bf16, tag="h2",
                                     name=f"h2_{G}_{band}")
                    nc.vector.tensor_scalar(
                        h2[:], pk[i][:], bb_sb[:, 4 * g + band:4 * g + band + 1],
                        0.0, Add, Max,
                    )
                    h2s[(G, band)] = h2

            def emit_l3_quad(G):
                w, g = divmod(G, NGW)
                if g == 0:
                    pos[w] = ps3.tile([128, WN], f32, tag="po", name=f"po_{w}")
                po = pos[w]
                for band in range(NBAND):
                    t = 4 * g + band
                    nc.tensor.matmul(
                        po[32 * band:32 * band + OUT_DIM, :],
                        w3p_sb[:, t * OUT_DIM:(t + 1) * OUT_DIM],
                        h2s.pop((G, band))[:],
                        start=(g == 0),
                        stop=(g == NGW - 1),
                        skip_group_check=True,
                        tile_position=(0, 32 * band),
                    )
                if g == NGW - 1:
                    # band merge: only one PSUM operand per DVE op, so chain
                    # SBUF accumulators across the four bands (+ b3).
                    po = pos.pop(w)
                    t1 = opool.tile([OUT_DIM, WN], f32, tag="t1", name=f"t1_{w}")
                    nc.vector.tensor_scalar_add(t1[:], po[0:OUT_DIM, :], b3_sb[:])
                    t2 = opool.tile([OUT_DIM, WN], f32, tag="t2", name=f"t2_{w}")
                    nc.vector.tensor_tensor(
                        t2[:], t1[:], po[32:32 + OUT_DIM, :], Add
                    )
                    t3 = opool.tile([OUT_DIM, WN], f32, tag="t3", name=f"t3_{w}")
                    nc.vector.tensor_tensor(
                        t3[:], t2[:], po[64:64 + OUT_DIM, :], Add
                    )
                    ot = opool.tile([OUT_DIM, WN], f32, tag="ot", name=f"ot_{w}")
                    nc.vector.tensor_tensor(
                        ot[:], t3[:], po[96:96 + OUT_DIM, :], Add
                    )
                    nc.sync.dma_start(outT[:, w * WN:(w + 1) * WN], ot[:])

            for G in range(NGTOT + 2):
                if G < NGTOT:
                    emit_l1_fulls(G, (0, 1))
                if 1 <= G <= NGTOT:
                    emit_l2_pack(G - 1, 0)
                if G < NGTOT:
                    emit_l1_fulls(G, (2, 3))
                if 1 <= G <= NGTOT:
                    emit_l2_pack(G - 1, 1)
                if G < NGTOT:
                    emit_l1_quad(G)
                if G >= 2:
                    emit_l3_quad(G - 2)

    nc.compile()
    return nc


def _get_program(bc=BC):
    if bc not in _PROGRAM_CACHE:
        _PROGRAM_CACHE[bc] = _build_program(bc)
    return _PROGRAM_CACHE[bc]


def _prep_weights(W1, b1, Wb, bb, W3, b3):
    """Host-side packing of replicated weights into device layouts."""
    W1 = np.asarray(W1, dtype=np.float32)
    Wb = np.asarray(Wb, dtype=np.float32)
    W3 = np.asarray(W3, dtype=np.float32)
    bb = np.asarray(bb, dtype=np.float32)

    # W1T [128, NM, K1, 128]: W1T[p, m, k, c] = W1.T[128k+p, 128m+c] -- the
    # per-partition-contiguous swizzle (1.5 KB DMA runs, per-m-tile DMA
    # granularity). W1L [128, 4096] holds the 16 leftover feature rows
    # replicated at partition offsets 0/32/64/96 for the row-group-packed
    # leftover matmuls.
    W1T = np.ascontiguousarray(
        W1.T[:K1 * 128].reshape(K1, 128, NM, 128).transpose(1, 2, 0, 3)
    ).astype(BF16)
    # W18 [128, NM8, 2, 128]: fp8 copy of chunks 0-1 for m-tiles < NM8,
    # scaled by SW8 (x carries 1/SW8, so products land at scale 1)
    W18 = np.ascontiguousarray(
        (W1.T[:2 * 128] * SW8).reshape(2, 128, NM, 128)[:, :, :NM8]
        .transpose(1, 2, 0, 3)
    ).astype(E4M3)
    W1L = np.zeros((128, HIDDEN), dtype=BF16)
    lo = W1.T[K1 * 128:IN_DIM].astype(BF16)
    for j in range(4):
        W1L[32 * j:32 * j + KL] = lo

    # Wb2 [128, NGW*128]: block (4*(4g+c)+r).T at [32r:+32, g*128+c*32:+32].
    Wb2 = np.zeros((128, NGW * 128), dtype=BF16)
    for g in range(NGW):
        for c in range(4):
            for r in range(4):
                blk = Wb[4 * (4 * g + c) + r].T.astype(BF16)  # [k, o]
                Wb2[32 * r:32 * r + 32,
                    g * 128 + c * 32:g * 128 + c * 32 + 32] = blk

    # Pack-output permutation: h2 band tile (g, band) partition 32c+o holds
    # hidden dim 128*(4g+c) + 32*band + o.
    # W3P [128, NM*10]: chunk t=4g+band at cols [t*10:+10]; W3P[p, t*10+o] =
    # W3[o, hid(g, band, p)].
    W3P = np.zeros((128, NM * OUT_DIM), dtype=BF16)
    bbP = np.zeros((128, 128), dtype=np.float32)
    for g in range(NGW):
        for band in range(4):
            t = 4 * g + band
            for c in range(4):
                hid0 = 128 * (4 * g + c) + 32 * band
                W3P[32 * c:32 * c + 32, t * OUT_DIM:(t + 1) * OUT_DIM] = (
                    W3[:, hid0:hid0 + 32].T.astype(BF16)
                )
                bbP[32 * c:32 * c + 32, t] = bb[hid0:hid0 + 32]

    NBC = NM + 128 + 1
    bcat = np.zeros((128, NBC), dtype=np.float32)
    bcat[:, 0:NM] = np.asarray(b1, np.float32).reshape(NM, 128).T
    bcat[:, NM:NM + 128] = bbP
    bcat[0:OUT_DIM, NM + 128] = np.asarray(b3, np.float32)
    return dict(W1T=W1T, W18=W18, W1L=W1L, Wb2=Wb2, W3P=W3P, bcat=bcat)


def _prep_x_shard(x, c, ncores=NCORES, bc=BC):
    xf = np.asarray(x[c * bc:(c + 1) * bc], dtype=np.float32).T  # [784, bc]
    xs = xf.astype(BF16)
    nw = bc // WN
    # xT [128, nw, K1, WN]: xT[p, w, k, b] = x.T[128k+p, 512w+b]
    xT = np.ascontiguousarray(
        xs[:K1 * 128].reshape(K1, 128, nw, WN).transpose(1, 2, 0, 3)
    )
    # X8 [128, nw, 2, WN]: fp8 copy of chunks 0-1, scaled by 1/SW8
    X8 = np.ascontiguousarray(
        (xf[:2 * 128] / SW8).reshape(2, 128, nw, WN).transpose(1, 2, 0, 3)
    ).astype(E4M3)
    xLs = np.zeros((128, bc), dtype=BF16)
    for j in range(4):
        xLs[32 * j:32 * j + KL] = xs[K1 * 128:IN_DIM]
    return xT, X8, xLs


def run(x, W1, b1, Wb, bb, W3, b3, trace=False, tmpdir=None):
    """Run on 8 cores; returns (out [B,10] fp32, BassKernelResults)."""
    from concourse.bass_utils import run_bass_kernel_spmd

    nc = _get_program()
    wmap = _prep_weights(W1, b1, Wb, bb, W3, b3)
    in_maps = []
    for c in range(NCORES):
        m = dict(wmap)
        m["xT"], m["X8"], m["xL"] = _prep_x_shard(np.asarray(x), c)
        in_maps.append(m)

    res = run_bass_kernel_spmd(
        nc, in_maps, core_ids=list(range(NCORES)), trace=trace, tmpdir=tmpdir
    )
    out = np.concatenate(
        [np.asarray(r["outT"]).T for r in res.results], axis=0
    ).astype(np.float32)
    return out, res


def kernel(x, W1, b1, Wb, bb, W3, b3):
    out, _ = run(x, W1, b1, Wb, bb, W3, b3, trace=False)
    return out


# revision 28
# speedup vs baseline: 1.0690x; 1.0194x over previous
"""Trainium2 Bass kernel for BlockDiagMNIST MLP.

Reference computation (all fp32):
    h  = relu(x @ W1.T + b1)          x:[B,784], W1:[4096,784]    -> [B,4096]
    yb = blockdiag(h, Wb)             Wb:[128,32,32] (h2[b, 32n+o] = sum_k h[b,32n+k] Wb[n,o,k])
    h2 = relu(yb + bb)
    out = h2 @ W3.T + b3              W3:[10,4096]                -> [B,10]

Strategy: pure data-parallel over batch (B=32768 -> 4096 rows/core on 8 cores),
weights replicated.  All matmuls in bf16 (fp32 PSUM accumulation, fp32 biases).
On-chip layout is transposed ("hidden on partitions"): we compute
hT = W1 @ x.T per 512-column batch window.

Layer 2 (block-diagonal) runs as packs of eight concurrent 32x32 PE-array
tiles (tile_position row x col grid): tile (32r, 32c) applies one diagonal
block to h-slice [32r:32r+32] of m-tile (4g+c), writing psum partitions
[32c:32c+32].  Two packs (block rows 0,1 then 2,3) cover a 4-m-tile group
using 2 PSUM banks; outputs land hidden-permuted, which the host-side bb/W3
packing compensates.

Layer 3 (M=10) runs as four concurrent column-tiles: K-chunk t accumulates
into psum partitions [32*(t%4) : +10]; a final DVE pass sums the four bands
(cross-quadrant reads) and adds b3.

Host-side prep (free -- not on the device timeline): transpose + bf16-cast of
x and weights, block/bias/W3 permutation packing.
"""

import numpy as np
import ml_dtypes

B = 32768
IN_DIM = 784
HIDDEN = 4096
BLOCK = 32
NUM_BLOCKS = 128
OUT_DIM = 10
NCORES = 8
BC = B // NCORES          # batch rows per core (4096)
WN = 512                  # batch-window columns (one matmul free-dim)
K1 = 6                    # full 128-row K-chunks for layer 1 (features 0..767)
KL = 16                   # leftover K rows (features 768..783), row-group packed
NM = HIDDEN // 128        # 32 hidden tiles per window
NGW = NM // 4             # 8 groups of 4 m-tiles per window
NBAND = 4                 # layer-3 column-tile bands
NM8 = 20                  # m-tiles whose first 2 K-chunks run in fp8 DoubleRow
SW8 = 32.0                # fp8 weight scale (x scaled by 1/SW8: product scale 1)

BF16 = ml_dtypes.bfloat16
E4M3 = ml_dtypes.float8_e4m3   # TRN FP8_EXP4-compatible (max +-240)

_PROGRAM_CACHE = {}


def _build_program(bc=BC):
    """Build (and bacc-compile) the per-core Bass program. bc = batch cols/core."""
    import concourse.mybir as mybir
    import concourse.tile as tile
    from concourse import bacc

    nw = bc // WN
    f32, bf16 = mybir.dt.float32, mybir.dt.bfloat16
    fp8 = mybir.dt.float8e4

    nc = bacc.Bacc("TRN2", target_bir_lowering=False, debug=False)

    # x / W1 stored pre-swizzled for per-partition contiguity (3 KB runs):
    # xT[p, w, k, b], W1T[p, j, k, m]
    xT = nc.dram_tensor("xT", [128, nw, K1, WN], bf16, kind="ExternalInput").ap()
    x8d = nc.dram_tensor("X8", [128, nw, 2, WN], fp8, kind="ExternalInput").ap()
    xL = nc.dram_tensor("xL", [128, bc], bf16, kind="ExternalInput").ap()
    w1t = nc.dram_tensor(
        "W1T", [128, NM, K1, 128], bf16, kind="ExternalInput"
    ).ap()
    w18d = nc.dram_tensor(
        "W18", [128, NM8, 2, 128], fp8, kind="ExternalInput"
    ).ap()
    w1l = nc.dram_tensor("W1L", [128, HIDDEN], bf16, kind="ExternalInput").ap()
    # Wb2: block (4*(4g+c)+r).T at [32r:32r+32, g*128+c*32 : +32]
    wb2 = nc.dram_tensor("Wb2", [128, NGW * 128], bf16, kind="ExternalInput").ap()
    # W3P: chunk t=4g+band -> [128, 10] at cols t*10; rows permuted to match the
    # pack layout (partition 32c+o of band tile <-> hidden 128*(4g+c)+32*band+o)
    w3p = nc.dram_tensor("W3P", [128, NM * OUT_DIM], bf16, kind="ExternalInput").ap()
    # biases packed into one tensor: cols 0..NM-1 = b1, NM..NM+127 = bbP
    # (col NM+4g+band, permuted like the L2 pack output), col NM+128 = b3
    NBC = NM + 128 + 1
    bcat = nc.dram_tensor("bcat", [128, NBC], f32, kind="ExternalInput").ap()
    outT = nc.dram_tensor("outT", [OUT_DIM, bc], f32, kind="ExternalOutput").ap()

    Relu = mybir.ActivationFunctionType.Relu
    Add = mybir.AluOpType.add
    Max = mybir.AluOpType.max

    MB = 4          # W1T column-block = MB m-tiles (DMA granularity for overlap)
    NJ = NM // MB   # 8 column blocks

    with tile.TileContext(nc) as tc:
        with (
            tc.tile_pool(name="const", bufs=1) as cpool,
            tc.tile_pool(name="xin", bufs=3) as xpool,
            tc.tile_pool(name="hbuf", bufs=10) as hpool,
            tc.tile_pool(name="h2buf", bufs=10) as h2pool,
            tc.tile_pool(name="obuf", bufs=4) as opool,
            tc.tile_pool(name="ps1", bufs=5, space="PSUM") as ps1,
            tc.tile_pool(name="ps2", bufs=2, space="PSUM") as ps2,
            tc.tile_pool(name="ps3", bufs=1, space="PSUM") as ps3,
        ):
            # HAM warmup on an un-DMA'd (garbage) SBUF tile: no data
            # dependency, so the PE clock gate starts ramping at t~0 and the
            # warmup stream covers the DMA-launch latency until real x/W1
            # data lands. Values are irrelevant (dummy psum, never read).
            gsb = cpool.tile([128, WN], bf16, name="warm_src")
            nc.vector.memset(gsb[:], 0)
            pw = ps2.tile([65, WN], f32, tag="p2", name="pwarm")
            for _ in range(20):
                nc.tensor.matmul(
                    pw[:, 0:65], gsb[:, 0:65], gsb[:, 0:65],
                    start=True, stop=True,
                )
            for _ in range(5):
                nc.tensor.matmul(
                    pw[:], gsb[:, 0:65], gsb[:],
                    start=True, stop=True,
                )

            def load_xt(w):
                """Per-window x tiles: two k-half DMAs (parallel queues, so the
                first window's data lands sooner) + the leftover rows."""
                KH = K1 // 2
                t8 = xpool.tile([128, 2, WN], fp8, tag="x8", name=f"x8_{w}")
                nc.sync.dma_start(t8[:], x8d[:, w, :, :])
                ta = xpool.tile([128, KH, WN], bf16, tag="xta", name=f"xta_{w}")
                nc.sync.dma_start(ta[:], xT[:, w, 0:KH, :])
                tb = xpool.tile([128, K1 - KH, WN], bf16, tag="xtb", name=f"xtb_{w}")
                nc.sync.dma_start(tb[:], xT[:, w, KH:K1, :])
                tl = xpool.tile([128, WN], bf16, tag="xl", name=f"xl_{w}")
                if w > 0:
                    nc.sync.dma_start(tl[:], xL[:, w * WN:(w + 1) * WN])
                return (ta, tb, t8), tl

            # Window-0 x tile before everything else so PE starts early.
            xts = {0: load_xt(0)}

            # Small constants (ACT/DVE need them by the first relu).
            bc_sb = cpool.tile([128, NBC], f32)
            nc.sync.dma_start(bc_sb[:], bcat)
            b1_sb = bc_sb[:, 0:NM]
            bb_sb = bc_sb[:, NM:NM + 128]
            b3_sb = bc_sb[0:OUT_DIM, NM + 128:NBC]

            # W1T as NM per-m-tile DMAs [128, K1, 128] (196 KB each): the
            # startup-critical chain is just x window 0 + m-tile 0's weights,
            # so real matmuls start as soon as ~700 KB has landed. The
            # leftover/L2/L3 weights are split per group and interleaved so
            # each group's full weight set arrives just ahead of its compute.
            w1l_sb = cpool.tile([128, HIDDEN], bf16)
            wb2_sb = cpool.tile([128, NGW * 128], bf16)
            w3p_sb = cpool.tile([128, NM * OUT_DIM], bf16)
            # DMA packet discipline: keep per-partition contiguous runs >=1KB.
            # w18 loads per group of 4 m-tiles (1KB/partition), w1l per group
            # (1KB/partition); wb2 (2KB/part) and w3p (640B/part) load whole.
            w18_sb = cpool.tile([128, NM8, 2, 128], fp8, name="w18")
            w18_t = [
                w18_sb[:, m, :, :] for m in range(NM8)
            ]
            nc.sync.dma_start(w18_sb[:, 0:4, :, :], w18d[:, 0:4, :, :])
            w1t_t = [None] * NM
            for m in range(NM):
                if m < NM8:
                    # fp8 m-tile: bf16 weights only for chunks 2..5
                    t = cpool.tile([128, K1 - 2, 128], bf16, name=f"w1m_{m}")
                    nc.sync.dma_start(t[:], w1t[:, m, 2:K1, :])
                else:
                    t = cpool.tile([128, K1, 128], bf16, name=f"w1m_{m}")
                    nc.sync.dma_start(t[:], w1t[:, m, :, :])
                w1t_t[m] = t
                if m == 3:
                    # window-0 leftover rows (needed at group 0's quad)
                    nc.sync.dma_start(xts[0][1][:], xL[:, 0:WN])
                if m % 4 == 3:
                    g = m // 4
                    nc.sync.dma_start(
                        w1l_sb[:, g * WN:(g + 1) * WN],
                        w1l[:, g * WN:(g + 1) * WN],
                    )
                    g8 = g + 1
                    if g8 * 4 < NM8:
                        hi = min((g8 + 1) * 4, NM8)
                        nc.sync.dma_start(
                            w18_sb[:, g8 * 4:hi, :, :], w18d[:, g8 * 4:hi, :, :]
                        )
                    if m == 3:
                        # L2/L3 weights: small heads arrive before the first
                        # packs; the bulk rides one packet-efficient DMA.
                        nc.sync.dma_start(
                            wb2_sb[:, 0:2 * 128], wb2[:, 0:2 * 128]
                        )
                        nc.sync.dma_start(w3p_sb[:], w3p)
                    if m == 7:
                        nc.sync.dma_start(
                            wb2_sb[:, 2 * 128:], wb2[:, 2 * 128:]
                        )

            # Software pipeline over G (global group index = window*NGW + g):
            #   L1(G) fulls interleaved with L2 packs of G-1 | L1 quad |
            #   L3 quad of G-2.
            NGTOT = nw * NGW
            pos = {}    # window -> psum accumulator for layer 3
            hs = {}     # G -> [4 h tiles]
            p1s_live = {}   # G -> [4 psum tiles] for the quad
            h2s = {}    # (G, band) -> h2 tile

            DR = mybir.MatmulPerfMode.DoubleRow

            def emit_l1_fulls(G, js):
                w, g = divmod(G, NGW)
                if g == 0 and js[0] == 0 and w not in xts:
                    xts[w] = load_xt(w)
                xt, _ = xts[w]
                ps = p1s_live.setdefault(G, [None] * 4)
                KH = K1 // 2
                for j in js:
                    m = 4 * g + j
                    p1 = ps1.tile([128, WN], f32, tag="p1", name=f"p1_{G}_{j}")
                    if m < NM8:
                        # chunks 0-1 as one fp8 DoubleRow matmul
                        nc.tensor.matmul(
                            p1[:],
                            w18_t[m][:],
                            xt[2][:],
                            start=True,
                            stop=False,
                            perf_mode=DR,
                        )
                        krange = range(2, K1)
                        woff = 2
                    else:
                        krange = range(K1)
                        woff = 0
                    for k in krange:
                        hi = k >= KH
                        nc.tensor.matmul(
                            p1[:],
                            w1t_t[m][:, k - woff, :],
                            xt[hi][:, k - KH * hi, :],
                            start=(k == 0 and woff == 0),
                            stop=False,
                        )
                    ps[j] = p1

            def emit_l1_quad(G):
                w, g = divmod(G, NGW)
                _, xl = xts[w]
                ps = p1s_live.pop(G)
                for j in range(4):
                    m = 4 * g + j
                    nc.tensor.matmul(
                        ps[j][:],
                        w1l_sb[32 * j:32 * j + KL, m * 128:(m + 1) * 128],
                        xl[32 * j:32 * j + KL, :],
                        start=False,
                        stop=True,
                        tile_position=(32 * j, 0),
                    )
                hts = []
                for j in range(4):
                    m = 4 * g + j
                    h = hpool.tile([128, WN], bf16, tag="h", name=f"h_{G}_{j}")
                    nc.scalar.activation(h[:], ps[j][:], Relu, bias=b1_sb[:, m:m + 1])
                    hts.append(h)
                hs[G] = hts

            def emit_l2_pack(G, half):
                """8 concurrent 32x32 tiles: block rows (2*half, 2*half+1) of
                the 4 m-tiles of group G, into 2 psum banks."""
                w, g = divmod(G, NGW)
                hts = hs[G]
                pk = [
                    ps2.tile([128, WN], f32, tag="p2", name=f"p2_{G}_{half}_{i}")
                    for i in range(2)
                ]
                for c in range(4):
                    for i in range(2):
                        r = 2 * half + i
                        nc.tensor.matmul(
                            pk[i][32 * c:32 * c + 32, :],
                            wb2_sb[32 * r:32 * r + 32,
                                   g * 128 + c * 32:g * 128 + c * 32 + 32],
                            hts[c][32 * r:32 * r + 32, :],
                            start=True,
                            stop=True,
                            tile_position=(32 * r, 32 * c),
                        )
                if half == 1:
                    hs.pop(G)
                for i in range(2):
                    band = 2 * half + i
                    h2 = h2pool.tile([128, WN], bf16, tag="h2",
                                     name=f"h2_{G}_{band}")
                    nc.vector.tensor_scalar(
                        h2[:], pk[i][:], bb_sb[:, 4 * g + band:4 * g + band + 1],
                        0.0, Add, Max,
                    )
                    h2s[(G, band)] = h2

            def emit_l3_quad(G):
                w, g = divmod(G, NGW)
                if g == 0:
                    pos[w] = ps3.tile([128, WN], f32, tag="po", name=f"po_{w}")
                po = pos[w]
                for band in range(NBAND):
                    t = 4 * g + band
                    nc.tensor.matmul(
                        po[32 * band:32 * band + OUT_DIM, :],
                        w3p_sb[:, t * OUT_DIM:(t + 1) * OUT_DIM],
                        h2s.pop((G, band))[:],
                        start=(g == 0),
                        stop=(g == NGW - 1),
                        skip_group_check=True,
                        tile_position=(0, 32 * band),
                    )
                if g == NGW - 1:
                    # band merge: only one PSUM operand per DVE op, so chain
                    # SBUF accumulators across the four bands (+ b3).
                    po = pos.pop(w)
                    t1 = opool.tile([OUT_DIM, WN], f32, tag="t1", name=f"t1_{w}")
                    nc.vector.tensor_scalar_add(t1[:], po[0:OUT_DIM, :], b3_sb[:])
                    t2 = opool.tile([OUT_DIM, WN], f32, tag="t2", name=f"t2_{w}")
                    nc.vector.tensor_tensor(
                        t2[:], t1[:], po[32:32 + OUT_DIM, :], Add
                    )
                    t3 = opool.tile([OUT_DIM, WN], f32, tag="t3", name=f"t3_{w}")
                    nc.vector.tensor_tensor(
                        t3[:], t2[:], po[64:64 + OUT_DIM, :], Add
                    )
                    ot = opool.tile([OUT_DIM, WN], f32, tag="ot", name=f"ot_{w}")
                    nc.vector.tensor_tensor(
                        ot[:], t3[:], po[96:96 + OUT_DIM, :], Add
                    )
                    nc.sync.dma_start(outT[:, w * WN:(w + 1) * WN], ot[:])

            for G in range(NGTOT + 2):
                if G < NGTOT:
                    emit_l1_fulls(G, (0, 1))
                if 1 <= G <= NGTOT:
                    emit_l2_pack(G - 1, 0)
                if G < NGTOT:
                    emit_l1_fulls(G, (2, 3))
                if 1 <= G <= NGTOT:
                    emit_l2_pack(G - 1, 1)
                if G < NGTOT:
                    emit_l1_quad(G)
                if G >= 2:
                    emit_l3_quad(G - 2)

    nc.compile()
    return nc


def _get_program(bc=BC):
    if bc not in _PROGRAM_CACHE:
        _PROGRAM_CACHE[bc] = _build_program(bc)
    return _PROGRAM_CACHE[bc]


def _prep_weights(W1, b1, Wb, bb, W3, b3):
    """Host-side packing of replicated weights into device layouts."""
    W1 = np.asarray(W1, dtype=np.float32)
    Wb = np.asarray(Wb, dtype=np.float32)
    W3 = np.asarray(W3, dtype=np.float32)
    bb = np.asarray(bb, dtype=np.float32)

    # W1T [128, NM, K1, 128]: W1T[p, m, k, c] = W1.T[128k+p, 128m+c] -- the
    # per-partition-contiguous swizzle (1.5 KB DMA runs, per-m-tile DMA
    # granularity). W1L [128, 4096] holds the 16 leftover feature rows
    # replicated at partition offsets 0/32/64/96 for the row-group-packed
    # leftover matmuls.
    W1T = np.ascontiguousarray(
        W1.T[:K1 * 128].reshape(K1, 128, NM, 128).transpose(1, 2, 0, 3)
    ).astype(BF16)
    # W18 [128, NM8, 2, 128]: fp8 copy of chunks 0-1 for m-tiles < NM8,
    # scaled by SW8 (x carries 1/SW8, so products land at scale 1)
    W18 = np.ascontiguousarray(
        (W1.T[:2 * 128] * SW8).reshape(2, 128, NM, 128)[:, :, :NM8]
        .transpose(1, 2, 0, 3)
    ).astype(E4M3)
    W1L = np.zeros((128, HIDDEN), dtype=BF16)
    lo = W1.T[K1 * 128:IN_DIM].astype(BF16)
    for j in range(4):
        W1L[32 * j:32 * j + KL] = lo

    # Wb2 [128, NGW*128]: block (4*(4g+c)+r).T at [32r:+32, g*128+c*32:+32].
    Wb2 = np.zeros((128, NGW * 128), dtype=BF16)
    for g in range(NGW):
        for c in range(4):
            for r in range(4):
                blk = Wb[4 * (4 * g + c) + r].T.astype(BF16)  # [k, o]
                Wb2[32 * r:32 * r + 32,
                    g * 128 + c * 32:g * 128 + c * 32 + 32] = blk

    # Pack-output permutation: h2 band tile (g, band) partition 32c+o holds
    # hidden dim 128*(4g+c) + 32*band + o.
    # W3P [128, NM*10]: chunk t=4g+band at cols [t*10:+10]; W3P[p, t*10+o] =
    # W3[o, hid(g, band, p)].
    W3P = np.zeros((128, NM * OUT_DIM), dtype=BF16)
    bbP = np.zeros((128, 128), dtype=np.float32)
    for g in range(NGW):
        for band in range(4):
            t = 4 * g + band
            for c in range(4):
                hid0 = 128 * (4 * g + c) + 32 * band
                W3P[32 * c:32 * c + 32, t * OUT_DIM:(t + 1) * OUT_DIM] = (
                    W3[:, hid0:hid0 + 32].T.astype(BF16)
                )
                bbP[32 * c:32 * c + 32, t] = bb[hid0:hid0 + 32]

    NBC = NM + 128 + 1
    bcat = np.zeros((128, NBC), dtype=np.float32)
    bcat[:, 0:NM] = np.asarray(b1, np.float32).reshape(NM, 128).T
    bcat[:, NM:NM + 128] = bbP
    bcat[0:OUT_DIM, NM + 128] = np.asarray(b3, np.float32)
    return dict(W1T=W1T, W18=W18, W1L=W1L, Wb2=Wb2, W3P=W3P, bcat=bcat)


def _prep_x_shard(x, c, ncores=NCORES, bc=BC):
    xf = np.asarray(x[c * bc:(c + 1) * bc], dtype=np.float32).T  # [784, bc]
    xs = xf.astype(BF16)
    nw = bc // WN
    # xT [128, nw, K1, WN]: xT[p, w, k, b] = x.T[128k+p, 512w+b]
    xT = np.ascontiguousarray(
        xs[:K1 * 128].reshape(K1, 128, nw, WN).transpose(1, 2, 0, 3)
    )
    # X8 [128, nw, 2, WN]: fp8 copy of chunks 0-1, scaled by 1/SW8
    X8 = np.ascontiguousarray(
        (xf[:2 * 128] / SW8).reshape(2, 128, nw, WN).transpose(1, 2, 0, 3)
    ).astype(E4M3)
    xLs = np.zeros((128, bc), dtype=BF16)
    for j in range(4):
        xLs[32 * j:32 * j + KL] = xs[K1 * 128:IN_DIM]
    return xT, X8, xLs


def run(x, W1, b1, Wb, bb, W3, b3, trace=False, tmpdir=None):
    """Run on 8 cores; returns (out [B,10] fp32, BassKernelResults)."""
    from concourse.bass_utils import run_bass_kernel_spmd

    nc = _get_program()
    wmap = _prep_weights(W1, b1, Wb, bb, W3, b3)
    in_maps = []
    for c in range(NCORES):
        m = dict(wmap)
        m["xT"], m["X8"], m["xL"] = _prep_x_shard(np.asarray(x), c)
        in_maps.append(m)

    res = run_bass_kernel_spmd(
        nc, in_maps, core_ids=list(range(NCORES)), trace=trace, tmpdir=tmpdir
    )
    out = np.concatenate(
        [np.asarray(r["outT"]).T for r in res.results], axis=0
    ).astype(np.float32)
    return out, res


def kernel(x, W1, b1, Wb, bb, W3, b3):
    out, _ = run(x, W1, b1, Wb, bb, W3, b3, trace=False)
    return out


# revision 31
# speedup vs baseline: 1.0720x; 1.0028x over previous
"""Trainium2 Bass kernel for BlockDiagMNIST MLP.

Reference computation (all fp32):
    h  = relu(x @ W1.T + b1)          x:[B,784], W1:[4096,784]    -> [B,4096]
    yb = blockdiag(h, Wb)             Wb:[128,32,32] (h2[b, 32n+o] = sum_k h[b,32n+k] Wb[n,o,k])
    h2 = relu(yb + bb)
    out = h2 @ W3.T + b3              W3:[10,4096]                -> [B,10]

Strategy: pure data-parallel over batch (B=32768 -> 4096 rows/core on 8 cores),
weights replicated.  All matmuls in bf16 (fp32 PSUM accumulation, fp32 biases).
On-chip layout is transposed ("hidden on partitions"): we compute
hT = W1 @ x.T per 512-column batch window.

Layer 2 (block-diagonal) runs as packs of eight concurrent 32x32 PE-array
tiles (tile_position row x col grid): tile (32r, 32c) applies one diagonal
block to h-slice [32r:32r+32] of m-tile (4g+c), writing psum partitions
[32c:32c+32].  Two packs (block rows 0,1 then 2,3) cover a 4-m-tile group
using 2 PSUM banks; outputs land hidden-permuted, which the host-side bb/W3
packing compensates.

Layer 3 (M=10) runs as four concurrent column-tiles: K-chunk t accumulates
into psum partitions [32*(t%4) : +10]; a final DVE pass sums the four bands
(cross-quadrant reads) and adds b3.

Host-side prep (free -- not on the device timeline): transpose + bf16-cast of
x and weights, block/bias/W3 permutation packing.
"""

import numpy as np
import ml_dtypes

B = 32768
IN_DIM = 784
HIDDEN = 4096
BLOCK = 32
NUM_BLOCKS = 128
OUT_DIM = 10
NCORES = 8
BC = B // NCORES          # batch rows per core (4096)
WN = 512                  # batch-window columns (one matmul free-dim)
K1 = 6                    # full 128-row K-chunks for layer 1 (features 0..767)
KL = 16                   # leftover K rows (features 768..783), row-group packed
NM = HIDDEN // 128        # 32 hidden tiles per window
NGW = NM // 4             # 8 groups of 4 m-tiles per window
NBAND = 4                 # layer-3 column-tile bands
NM8 = 20                  # m-tiles whose first 2 K-chunks run in fp8 DoubleRow
SW8 = 32.0                # fp8 weight scale (x scaled by 1/SW8: product scale 1)

BF16 = ml_dtypes.bfloat16
E4M3 = ml_dtypes.float8_e4m3   # TRN FP8_EXP4-compatible (max +-240)

_PROGRAM_CACHE = {}


def _build_program(bc=BC):
    """Build (and bacc-compile) the per-core Bass program. bc = batch cols/core."""
    import concourse.mybir as mybir
    import concourse.tile as tile
    from concourse import bacc

    nw = bc // WN
    f32, bf16 = mybir.dt.float32, mybir.dt.bfloat16
    fp8 = mybir.dt.float8e4

    nc = bacc.Bacc("TRN2", target_bir_lowering=False, debug=False)

    # x / W1 stored pre-swizzled for per-partition contiguity (3 KB runs):
    # xT[p, w, k, b], W1T[p, j, k, m]
    xT = nc.dram_tensor("xT", [128, nw, K1, WN], bf16, kind="ExternalInput").ap()
    x8d = nc.dram_tensor("X8", [128, nw, 2, WN], fp8, kind="ExternalInput").ap()
    xL = nc.dram_tensor("xL", [128, bc], bf16, kind="ExternalInput").ap()
    w1t = nc.dram_tensor(
        "W1T", [128, NM, K1, 128], bf16, kind="ExternalInput"
    ).ap()
    w18d = nc.dram_tensor(
        "W18", [128, NM8, 2, 128], fp8, kind="ExternalInput"
    ).ap()
    w1l = nc.dram_tensor("W1L", [128, HIDDEN], bf16, kind="ExternalInput").ap()
    # Wb2: block (4*(4g+c)+r).T at [32r:32r+32, g*128+c*32 : +32]
    wb2 = nc.dram_tensor("Wb2", [128, NGW * 128], bf16, kind="ExternalInput").ap()
    # W3P: chunk t=4g+band -> [128, 10] at cols t*10; rows permuted to match the
    # pack layout (partition 32c+o of band tile <-> hidden 128*(4g+c)+32*band+o)
    w3p = nc.dram_tensor("W3P", [128, NM * OUT_DIM], bf16, kind="ExternalInput").ap()
    # biases packed into one tensor: cols 0..NM-1 = b1, NM..NM+127 = bbP
    # (col NM+4g+band, permuted like the L2 pack output), col NM+128 = b3
    NBC = NM + 128 + 1
    bcat = nc.dram_tensor("bcat", [128, NBC], f32, kind="ExternalInput").ap()
    outT = nc.dram_tensor("outT", [OUT_DIM, bc], f32, kind="ExternalOutput").ap()

    Relu = mybir.ActivationFunctionType.Relu
    Add = mybir.AluOpType.add
    Max = mybir.AluOpType.max

    MB = 4          # W1T column-block = MB m-tiles (DMA granularity for overlap)
    NJ = NM // MB   # 8 column blocks

    with tile.TileContext(nc) as tc:
        with (
            tc.tile_pool(name="const", bufs=1) as cpool,
            tc.tile_pool(name="xin", bufs=3) as xpool,
            tc.tile_pool(name="hbuf", bufs=10) as hpool,
            tc.tile_pool(name="h2buf", bufs=10) as h2pool,
            tc.tile_pool(name="obuf", bufs=4) as opool,
            tc.tile_pool(name="ps1", bufs=5, space="PSUM") as ps1,
            tc.tile_pool(name="ps2", bufs=2, space="PSUM") as ps2,
            tc.tile_pool(name="ps3", bufs=1, space="PSUM") as ps3,
        ):
            # HAM warmup on an un-DMA'd (garbage) SBUF tile: no data
            # dependency, so the PE clock gate starts ramping at t~0 and the
            # warmup stream covers the DMA-launch latency until real x/W1
            # data lands. Values are irrelevant (dummy psum, never read).
            gsb = cpool.tile([128, WN], bf16, name="warm_src")
            nc.vector.memset(gsb[:], 0)
            pw = ps2.tile([65, WN], f32, tag="p2", name="pwarm")
            for _ in range(20):
                nc.tensor.matmul(
                    pw[:, 0:65], gsb[:, 0:65], gsb[:, 0:65],
                    start=True, stop=True,
                )
            for _ in range(12):
                nc.tensor.matmul(
                    pw[:], gsb[:, 0:65], gsb[:],
                    start=True, stop=True,
                )

            def load_xt(w):
                """Per-window x tiles: two k-half DMAs (parallel queues, so the
                first window's data lands sooner) + the leftover rows."""
                KH = K1 // 2
                t8 = xpool.tile([128, 2, WN], fp8, tag="x8", name=f"x8_{w}")
                nc.sync.dma_start(t8[:], x8d[:, w, :, :])
                ta = xpool.tile([128, KH, WN], bf16, tag="xta", name=f"xta_{w}")
                nc.sync.dma_start(ta[:], xT[:, w, 0:KH, :])
                tb = xpool.tile([128, K1 - KH, WN], bf16, tag="xtb", name=f"xtb_{w}")
                nc.sync.dma_start(tb[:], xT[:, w, KH:K1, :])
                tl = xpool.tile([128, WN], bf16, tag="xl", name=f"xl_{w}")
                if w > 0:
                    nc.sync.dma_start(tl[:], xL[:, w * WN:(w + 1) * WN])
                return (ta, tb, t8), tl

            # Window-0 x tile before everything else so PE starts early.
            xts = {0: load_xt(0)}

            # Small constants (ACT/DVE need them by the first relu).
            bc_sb = cpool.tile([128, NBC], f32)
            nc.sync.dma_start(bc_sb[:], bcat)
            b1_sb = bc_sb[:, 0:NM]
            bb_sb = bc_sb[:, NM:NM + 128]
            b3_sb = bc_sb[0:OUT_DIM, NM + 128:NBC]

            # W1T as NM per-m-tile DMAs [128, K1, 128] (196 KB each): the
            # startup-critical chain is just x window 0 + m-tile 0's weights,
            # so real matmuls start as soon as ~700 KB has landed. The
            # leftover/L2/L3 weights are split per group and interleaved so
            # each group's full weight set arrives just ahead of its compute.
            w1l_sb = cpool.tile([128, HIDDEN], bf16)
            wb2_sb = cpool.tile([128, NGW * 128], bf16)
            w3p_sb = cpool.tile([128, NM * OUT_DIM], bf16)
            # DMA packet discipline: keep per-partition contiguous runs >=1KB.
            # w18 loads per group of 4 m-tiles (1KB/partition), w1l per group
            # (1KB/partition); wb2 (2KB/part) and w3p (640B/part) load whole.
            w18_sb = cpool.tile([128, NM8, 2, 128], fp8, name="w18")
            w18_t = [
                w18_sb[:, m, :, :] for m in range(NM8)
            ]
            nc.sync.dma_start(w18_sb[:, 0:4, :, :], w18d[:, 0:4, :, :])
            w1t_t = [None] * NM
            for m in range(NM):
                if m < NM8:
                    # fp8 m-tile: bf16 weights only for chunks 2..5
                    t = cpool.tile([128, K1 - 2, 128], bf16, name=f"w1m_{m}")
                    nc.sync.dma_start(t[:], w1t[:, m, 2:K1, :])
                else:
                    t = cpool.tile([128, K1, 128], bf16, name=f"w1m_{m}")
                    nc.sync.dma_start(t[:], w1t[:, m, :, :])
                w1t_t[m] = t
                if m == 3:
                    # window-0 leftover rows (needed at group 0's quad)
                    nc.sync.dma_start(xts[0][1][:], xL[:, 0:WN])
                if m % 4 == 3:
                    g = m // 4
                    nc.sync.dma_start(
                        w1l_sb[:, g * WN:(g + 1) * WN],
                        w1l[:, g * WN:(g + 1) * WN],
                    )
                    g8 = g + 1
                    if g8 * 4 < NM8:
                        hi = min((g8 + 1) * 4, NM8)
                        nc.sync.dma_start(
                            w18_sb[:, g8 * 4:hi, :, :], w18d[:, g8 * 4:hi, :, :]
                        )
                    if m == 3:
                        # L2/L3 weights: small heads arrive before the first
                        # packs; the bulk rides one packet-efficient DMA.
                        nc.sync.dma_start(
                            wb2_sb[:, 0:2 * 128], wb2[:, 0:2 * 128]
                        )
                        nc.sync.dma_start(w3p_sb[:], w3p)
                    if m == 7:
                        nc.sync.dma_start(
                            wb2_sb[:, 2 * 128:], wb2[:, 2 * 128:]
                        )

            # Software pipeline over G (global group index = window*NGW + g):
            #   L1(G) fulls interleaved with L2 packs of G-1 | L1 quad |
            #   L3 quad of G-2.
            NGTOT = nw * NGW
            pos = {}    # window -> psum accumulator for layer 3
            hs = {}     # G -> [4 h tiles]
            p1s_live = {}   # G -> [4 psum tiles] for the quad
            h2s = {}    # (G, band) -> h2 tile

            DR = mybir.MatmulPerfMode.DoubleRowSwInterleave

            def emit_l1_fulls(G, js):
                w, g = divmod(G, NGW)
                if g == 0 and js[0] == 0 and w not in xts:
                    xts[w] = load_xt(w)
                xt, _ = xts[w]
                ps = p1s_live.setdefault(G, [None] * 4)
                KH = K1 // 2
                for j in js:
                    m = 4 * g + j
                    p1 = ps1.tile([128, WN], f32, tag="p1", name=f"p1_{G}_{j}")
                    if m < NM8:
                        # chunks 0-1 as one fp8 DoubleRow matmul
                        nc.tensor.matmul(
                            p1[:],
                            w18_t[m][:],
                            xt[2][:],
                            start=True,
                            stop=False,
                            perf_mode=DR,
                        )
                        krange = range(2, K1)
                        woff = 2
                    else:
                        krange = range(K1)
                        woff = 0
                    for k in krange:
                        hi = k >= KH
                        nc.tensor.matmul(
                            p1[:],
                            w1t_t[m][:, k - woff, :],
                            xt[hi][:, k - KH * hi, :],
                            start=(k == 0 and woff == 0),
                            stop=False,
                        )
                    ps[j] = p1

            def emit_l1_quad(G):
                w, g = divmod(G, NGW)
                _, xl = xts[w]
                ps = p1s_live.pop(G)
                for j in range(4):
                    m = 4 * g + j
                    nc.tensor.matmul(
                        ps[j][:],
                        w1l_sb[32 * j:32 * j + KL, m * 128:(m + 1) * 128],
                        xl[32 * j:32 * j + KL, :],
                        start=False,
                        stop=True,
                        tile_position=(32 * j, 0),
                    )
                hts = []
                for j in range(4):
                    m = 4 * g + j
                    h = hpool.tile([128, WN], bf16, tag="h", name=f"h_{G}_{j}")
                    nc.scalar.activation(h[:], ps[j][:], Relu, bias=b1_sb[:, m:m + 1])
                    hts.append(h)
                hs[G] = hts

            def emit_l2_pack(G, half):
                """8 concurrent 32x32 tiles: block rows (2*half, 2*half+1) of
                the 4 m-tiles of group G, into 2 psum banks."""
                w, g = divmod(G, NGW)
                hts = hs[G]
                pk = [
                    ps2.tile([128, WN], f32, tag="p2", name=f"p2_{G}_{half}_{i}")
                    for i in range(2)
                ]
                for c in range(4):
                    for i in range(2):
                        r = 2 * half + i
                        nc.tensor.matmul(
                            pk[i][32 * c:32 * c + 32, :],
                            wb2_sb[32 * r:32 * r + 32,
                                   g * 128 + c * 32:g * 128 + c * 32 + 32],
                            hts[c][32 * r:32 * r + 32, :],
                            start=True,
                            stop=True,
                            tile_position=(32 * r, 32 * c),
                        )
                if half == 1:
                    hs.pop(G)
                for i in range(2):
                    band = 2 * half + i
                    h2 = h2pool.tile([128, WN], bf16, tag="h2",
                                     name=f"h2_{G}_{band}")
                    nc.vector.tensor_scalar(
                        h2[:], pk[i][:], bb_sb[:, 4 * g + band:4 * g + band + 1],
                        0.0, Add, Max,
                    )
                    h2s[(G, band)] = h2

            def emit_l3_quad(G):
                w, g = divmod(G, NGW)
                if g == 0:
                    pos[w] = ps3.tile([128, WN], f32, tag="po", name=f"po_{w}")
                po = pos[w]
                for band in range(NBAND):
                    t = 4 * g + band
                    nc.tensor.matmul(
                        po[32 * band:32 * band + OUT_DIM, :],
                        w3p_sb[:, t * OUT_DIM:(t + 1) * OUT_DIM],
                        h2s.pop((G, band))[:],
                        start=(g == 0),
                        stop=(g == NGW - 1),
                        skip_group_check=True,
                        tile_position=(0, 32 * band),
                    )
                if g == NGW - 1:
                    # band merge: only one PSUM operand per DVE op, so chain
                    # SBUF accumulators across the four bands (+ b3).
                    po = pos.pop(w)
                    t1 = opool.tile([OUT_DIM, WN], f32, tag="t1", name=f"t1_{w}")
                    nc.vector.tensor_scalar_add(t1[:], po[0:OUT_DIM, :], b3_sb[:])
                    t2 = opool.tile([OUT_DIM, WN], f32, tag="t2", name=f"t2_{w}")
                    nc.vector.tensor_tensor(
                        t2[:], t1[:], po[32:32 + OUT_DIM, :], Add
                    )
                    t3 = opool.tile([OUT_DIM, WN], f32, tag="t3", name=f"t3_{w}")
                    nc.vector.tensor_tensor(
                        t3[:], t2[:], po[64:64 + OUT_DIM, :], Add
                    )
                    ot = opool.tile([OUT_DIM, WN], f32, tag="ot", name=f"ot_{w}")
                    nc.vector.tensor_tensor(
                        ot[:], t3[:], po[96:96 + OUT_DIM, :], Add
                    )
                    nc.sync.dma_start(outT[:, w * WN:(w + 1) * WN], ot[:])

            for G in range(NGTOT + 2):
                if G < NGTOT:
                    emit_l1_fulls(G, (0, 1))
                if 1 <= G <= NGTOT:
                    emit_l2_pack(G - 1, 0)
                if G < NGTOT:
                    emit_l1_fulls(G, (2, 3))
                if 1 <= G <= NGTOT:
                    emit_l2_pack(G - 1, 1)
                if G < NGTOT:
                    emit_l1_quad(G)
                if G >= 2:
                    emit_l3_quad(G - 2)

    nc.compile()
    return nc


def _get_program(bc=BC):
    if bc not in _PROGRAM_CACHE:
        _PROGRAM_CACHE[bc] = _build_program(bc)
    return _PROGRAM_CACHE[bc]


def _prep_weights(W1, b1, Wb, bb, W3, b3):
    """Host-side packing of replicated weights into device layouts."""
    W1 = np.asarray(W1, dtype=np.float32)
    Wb = np.asarray(Wb, dtype=np.float32)
    W3 = np.asarray(W3, dtype=np.float32)
    bb = np.asarray(bb, dtype=np.float32)

    # W1T [128, NM, K1, 128]: W1T[p, m, k, c] = W1.T[128k+p, 128m+c] -- the
    # per-partition-contiguous swizzle (1.5 KB DMA runs, per-m-tile DMA
    # granularity). W1L [128, 4096] holds the 16 leftover feature rows
    # replicated at partition offsets 0/32/64/96 for the row-group-packed
    # leftover matmuls.
    W1T = np.ascontiguousarray(
        W1.T[:K1 * 128].reshape(K1, 128, NM, 128).transpose(1, 2, 0, 3)
    ).astype(BF16)
    # W18 [128, NM8, 2, 128]: fp8 copy of chunks 0-1 for m-tiles < NM8,
    # scaled by SW8 (x carries 1/SW8, so products land at scale 1).
    # Stored in DoubleRowSwInterleave layout: flat free dim holds
    # [A127 B127 A126 B126 ... A0 B0] (A/B = chunk0/1 weight columns,
    # reversed order) so the HW weight load reads contiguously.
    W18L = (
        (W1.T[:2 * 128] * SW8).reshape(2, 128, NM, 128)[:, :, :NM8]
        .transpose(1, 2, 0, 3)
    ).astype(E4M3)  # logical [p, m, i, c]
    W18 = np.ascontiguousarray(
        W18L[:, :, :, ::-1].transpose(0, 1, 3, 2)
    ).reshape(128, NM8, 2, 128)
    W1L = np.zeros((128, HIDDEN), dtype=BF16)
    lo = W1.T[K1 * 128:IN_DIM].astype(BF16)
    for j in range(4):
        W1L[32 * j:32 * j + KL] = lo

    # Wb2 [128, NGW*128]: block (4*(4g+c)+r).T at [32r:+32, g*128+c*32:+32].
    Wb2 = np.zeros((128, NGW * 128), dtype=BF16)
    for g in range(NGW):
        for c in range(4):
            for r in range(4):
                blk = Wb[4 * (4 * g + c) + r].T.astype(BF16)  # [k, o]
                Wb2[32 * r:32 * r + 32,
                    g * 128 + c * 32:g * 128 + c * 32 + 32] = blk

    # Pack-output permutation: h2 band tile (g, band) partition 32c+o holds
    # hidden dim 128*(4g+c) + 32*band + o.
    # W3P [128, NM*10]: chunk t=4g+band at cols [t*10:+10]; W3P[p, t*10+o] =
    # W3[o, hid(g, band, p)].
    W3P = np.zeros((128, NM * OUT_DIM), dtype=BF16)
    bbP = np.zeros((128, 128), dtype=np.float32)
    for g in range(NGW):
        for band in range(4):
            t = 4 * g + band
            for c in range(4):
                hid0 = 128 * (4 * g + c) + 32 * band
                W3P[32 * c:32 * c + 32, t * OUT_DIM:(t + 1) * OUT_DIM] = (
                    W3[:, hid0:hid0 + 32].T.astype(BF16)
                )
                bbP[32 * c:32 * c + 32, t] = bb[hid0:hid0 + 32]

    NBC = NM + 128 + 1
    bcat = np.zeros((128, NBC), dtype=np.float32)
    bcat[:, 0:NM] = np.asarray(b1, np.float32).reshape(NM, 128).T
    bcat[:, NM:NM + 128] = bbP
    bcat[0:OUT_DIM, NM + 128] = np.asarray(b3, np.float32)
    return dict(W1T=W1T, W18=W18, W1L=W1L, Wb2=Wb2, W3P=W3P, bcat=bcat)


def _prep_x_shard(x, c, ncores=NCORES, bc=BC):
    xf = np.asarray(x[c * bc:(c + 1) * bc], dtype=np.float32).T  # [784, bc]
    xs = xf.astype(BF16)
    nw = bc // WN
    # xT [128, nw, K1, WN]: xT[p, w, k, b] = x.T[128k+p, 512w+b]
    xT = np.ascontiguousarray(
        xs[:K1 * 128].reshape(K1, 128, nw, WN).transpose(1, 2, 0, 3)
    )
    # X8 [128, nw, 2, WN]: fp8 copy of chunks 0-1, scaled by 1/SW8
    X8 = np.ascontiguousarray(
        (xf[:2 * 128] / SW8).reshape(2, 128, nw, WN).transpose(1, 2, 0, 3)
    ).astype(E4M3)
    xLs = np.zeros((128, bc), dtype=BF16)
    for j in range(4):
        xLs[32 * j:32 * j + KL] = xs[K1 * 128:IN_DIM]
    return xT, X8, xLs


def run(x, W1, b1, Wb, bb, W3, b3, trace=False, tmpdir=None):
    """Run on 8 cores; returns (out [B,10] fp32, BassKernelResults)."""
    from concourse.bass_utils import run_bass_kernel_spmd

    nc = _get_program()
    wmap = _prep_weights(W1, b1, Wb, bb, W3, b3)
    in_maps = []
    for c in range(NCORES):
        m = dict(wmap)
        m["xT"], m["X8"], m["xL"] = _prep_x_shard(np.asarray(x), c)
        in_maps.append(m)

    res = run_bass_kernel_spmd(
        nc, in_maps, core_ids=list(range(NCORES)), trace=trace, tmpdir=tmpdir
    )
    out = np.concatenate(
        [np.asarray(r["outT"]).T for r in res.results], axis=0
    ).astype(np.float32)
    return out, res


def kernel(x, W1, b1, Wb, bb, W3, b3):
    out, _ = run(x, W1, b1, Wb, bb, W3, b3, trace=False)
    return out


# revision 32
# speedup vs baseline: 1.0787x; 1.0062x over previous
"""Trainium2 Bass kernel for BlockDiagMNIST MLP.

Reference computation (all fp32):
    h  = relu(x @ W1.T + b1)          x:[B,784], W1:[4096,784]    -> [B,4096]
    yb = blockdiag(h, Wb)             Wb:[128,32,32] (h2[b, 32n+o] = sum_k h[b,32n+k] Wb[n,o,k])
    h2 = relu(yb + bb)
    out = h2 @ W3.T + b3              W3:[10,4096]                -> [B,10]

Strategy: pure data-parallel over batch (B=32768 -> 4096 rows/core on 8 cores),
weights replicated.  All matmuls in bf16 (fp32 PSUM accumulation, fp32 biases).
On-chip layout is transposed ("hidden on partitions"): we compute
hT = W1 @ x.T per 512-column batch window.

Layer 2 (block-diagonal) runs as packs of eight concurrent 32x32 PE-array
tiles (tile_position row x col grid): tile (32r, 32c) applies one diagonal
block to h-slice [32r:32r+32] of m-tile (4g+c), writing psum partitions
[32c:32c+32].  Two packs (block rows 0,1 then 2,3) cover a 4-m-tile group
using 2 PSUM banks; outputs land hidden-permuted, which the host-side bb/W3
packing compensates.

Layer 3 (M=10) runs as four concurrent column-tiles: K-chunk t accumulates
into psum partitions [32*(t%4) : +10]; a final DVE pass sums the four bands
(cross-quadrant reads) and adds b3.

Host-side prep (free -- not on the device timeline): transpose + bf16-cast of
x and weights, block/bias/W3 permutation packing.
"""

import numpy as np
import ml_dtypes

B = 32768
IN_DIM = 784
HIDDEN = 4096
BLOCK = 32
NUM_BLOCKS = 128
OUT_DIM = 10
NCORES = 8
BC = B // NCORES          # batch rows per core (4096)
WN = 512                  # batch-window columns (one matmul free-dim)
K1 = 6                    # full 128-row K-chunks for layer 1 (features 0..767)
KL = 16                   # leftover K rows (features 768..783), row-group packed
NM = HIDDEN // 128        # 32 hidden tiles per window
NGW = NM // 4             # 8 groups of 4 m-tiles per window
NBAND = 4                 # layer-3 column-tile bands
NM8 = 20                  # m-tiles whose first 2 K-chunks run in fp8 DoubleRow
SW8 = 32.0                # fp8 weight scale (x scaled by 1/SW8: product scale 1)

BF16 = ml_dtypes.bfloat16
E4M3 = ml_dtypes.float8_e4m3   # TRN FP8_EXP4-compatible (max +-240)

_PROGRAM_CACHE = {}


def _build_program(bc=BC):
    """Build (and bacc-compile) the per-core Bass program. bc = batch cols/core."""
    import concourse.mybir as mybir
    import concourse.tile as tile
    from concourse import bacc

    nw = bc // WN
    f32, bf16 = mybir.dt.float32, mybir.dt.bfloat16
    fp8 = mybir.dt.float8e4

    nc = bacc.Bacc("TRN2", target_bir_lowering=False, debug=False)

    # x / W1 stored pre-swizzled for per-partition contiguity (3 KB runs):
    # xT[p, w, k, b], W1T[p, j, k, m]
    xT = nc.dram_tensor("xT", [128, nw, K1, WN], bf16, kind="ExternalInput").ap()
    x8d = nc.dram_tensor("X8", [128, nw, 2, WN], fp8, kind="ExternalInput").ap()
    xL = nc.dram_tensor("xL", [128, bc], bf16, kind="ExternalInput").ap()
    w1t = nc.dram_tensor(
        "W1T", [128, NM, K1, 128], bf16, kind="ExternalInput"
    ).ap()
    w18d = nc.dram_tensor(
        "W18", [128, NM8, 2, 128], fp8, kind="ExternalInput"
    ).ap()
    w1l = nc.dram_tensor("W1L", [128, HIDDEN], bf16, kind="ExternalInput").ap()
    # Wb2: block (4*(4g+c)+r).T at [32r:32r+32, g*128+c*32 : +32]
    wb2 = nc.dram_tensor("Wb2", [128, NGW * 128], bf16, kind="ExternalInput").ap()
    # W3P: chunk t=4g+band -> [128, 10] at cols t*10; rows permuted to match the
    # pack layout (partition 32c+o of band tile <-> hidden 128*(4g+c)+32*band+o)
    w3p = nc.dram_tensor("W3P", [128, NM * OUT_DIM], bf16, kind="ExternalInput").ap()
    # biases packed into one tensor: cols 0..NM-1 = b1, NM..NM+127 = bbP
    # (col NM+4g+band, permuted like the L2 pack output), col NM+128 = b3
    NBC = NM + 128 + 1
    bcat = nc.dram_tensor("bcat", [128, NBC], f32, kind="ExternalInput").ap()
    outT = nc.dram_tensor("outT", [OUT_DIM, bc], f32, kind="ExternalOutput").ap()

    Relu = mybir.ActivationFunctionType.Relu
    Add = mybir.AluOpType.add
    Max = mybir.AluOpType.max

    MB = 4          # W1T column-block = MB m-tiles (DMA granularity for overlap)
    NJ = NM // MB   # 8 column blocks

    with tile.TileContext(nc) as tc:
        with (
            tc.tile_pool(name="const", bufs=1) as cpool,
            tc.tile_pool(name="xin", bufs=3) as xpool,
            tc.tile_pool(name="hbuf", bufs=10) as hpool,
            tc.tile_pool(name="h2buf", bufs=10) as h2pool,
            tc.tile_pool(name="obuf", bufs=4) as opool,
            tc.tile_pool(name="ps1", bufs=5, space="PSUM") as ps1,
            tc.tile_pool(name="ps2", bufs=2, space="PSUM") as ps2,
            tc.tile_pool(name="ps3", bufs=1, space="PSUM") as ps3,
        ):
            # HAM warmup on an un-DMA'd (garbage) SBUF tile: no data
            # dependency, so the PE clock gate starts ramping at t~0 and the
            # warmup stream covers the DMA-launch latency until real x/W1
            # data lands. Values are irrelevant (dummy psum, never read).
            gsb = cpool.tile([128, WN], bf16, name="warm_src")
            nc.vector.memset(gsb[:], 0)
            pw = ps2.tile([65, WN], f32, tag="p2", name="pwarm")
            for _ in range(20):
                nc.tensor.matmul(
                    pw[:, 0:65], gsb[:, 0:65], gsb[:, 0:65],
                    start=True, stop=True,
                )
            for _ in range(12):
                nc.tensor.matmul(
                    pw[:], gsb[:, 0:65], gsb[:],
                    start=True, stop=True,
                )

            def load_xt(w):
                """Per-window x tiles: two k-half DMAs (parallel queues, so the
                first window's data lands sooner) + the leftover rows."""
                KH = K1 // 2
                t8 = xpool.tile([128, 2, WN], fp8, tag="x8", name=f"x8_{w}")
                nc.sync.dma_start(t8[:], x8d[:, w, :, :])
                ta = xpool.tile([128, KH, WN], bf16, tag="xta", name=f"xta_{w}")
                nc.sync.dma_start(ta[:], xT[:, w, 0:KH, :])
                tb = xpool.tile([128, K1 - KH, WN], bf16, tag="xtb", name=f"xtb_{w}")
                nc.sync.dma_start(tb[:], xT[:, w, KH:K1, :])
                tl = xpool.tile([128, WN], bf16, tag="xl", name=f"xl_{w}")
                if w > 0:
                    nc.sync.dma_start(tl[:], xL[:, w * WN:(w + 1) * WN])
                return (ta, tb, t8), tl

            # Window-0 x tile before everything else so PE starts early.
            xts = {0: load_xt(0)}

            # Small constants (ACT/DVE need them by the first relu).
            bc_sb = cpool.tile([128, NBC], f32)
            nc.sync.dma_start(bc_sb[:], bcat)
            b1_sb = bc_sb[:, 0:NM]
            bb_sb = bc_sb[:, NM:NM + 128]
            b3_sb = bc_sb[0:OUT_DIM, NM + 128:NBC]

            # W1T as NM per-m-tile DMAs [128, K1, 128] (196 KB each): the
            # startup-critical chain is just x window 0 + m-tile 0's weights,
            # so real matmuls start as soon as ~700 KB has landed. The
            # leftover/L2/L3 weights are split per group and interleaved so
            # each group's full weight set arrives just ahead of its compute.
            w1l_sb = cpool.tile([128, HIDDEN], bf16)
            wb2_sb = cpool.tile([128, NGW * 128], bf16)
            w3p_sb = cpool.tile([128, NM * OUT_DIM], bf16)
            # DMA packet discipline: keep per-partition contiguous runs >=1KB.
            # w18 loads per group of 4 m-tiles (1KB/partition), w1l per group
            # (1KB/partition); wb2 (2KB/part) and w3p (640B/part) load whole.
            w18_sb = cpool.tile([128, NM8, 2, 128], fp8, name="w18")
            w18_t = [
                w18_sb[:, m, :, :] for m in range(NM8)
            ]
            nc.sync.dma_start(w18_sb[:, 0:4, :, :], w18d[:, 0:4, :, :])
            w1t_t = [None] * NM
            for m in range(NM):
                if m < NM8:
                    # fp8 m-tile: bf16 weights only for chunks 2..5
                    t = cpool.tile([128, K1 - 2, 128], bf16, name=f"w1m_{m}")
                    nc.sync.dma_start(t[:], w1t[:, m, 2:K1, :])
                else:
                    t = cpool.tile([128, K1, 128], bf16, name=f"w1m_{m}")
                    nc.sync.dma_start(t[:], w1t[:, m, :, :])
                w1t_t[m] = t
                if m == 3:
                    # window-0 leftover rows (needed at group 0's quad)
                    nc.sync.dma_start(xts[0][1][:], xL[:, 0:WN])
                if m % 4 == 3:
                    g = m // 4
                    nc.sync.dma_start(
                        w1l_sb[:, g * WN:(g + 1) * WN],
                        w1l[:, g * WN:(g + 1) * WN],
                    )
                    g8 = g + 1
                    if g8 * 4 < NM8:
                        hi = min((g8 + 1) * 4, NM8)
                        nc.sync.dma_start(
                            w18_sb[:, g8 * 4:hi, :, :], w18d[:, g8 * 4:hi, :, :]
                        )
                    if m == 3:
                        # L2/L3 weights: small heads arrive before the first
                        # packs; the bulk rides one packet-efficient DMA.
                        nc.sync.dma_start(
                            wb2_sb[:, 0:2 * 128], wb2[:, 0:2 * 128]
                        )
                        nc.sync.dma_start(w3p_sb[:], w3p)
                    if m == 7:
                        nc.sync.dma_start(
                            wb2_sb[:, 2 * 128:], wb2[:, 2 * 128:]
                        )

            # Software pipeline over G (global group index = window*NGW + g):
            #   L1(G) fulls interleaved with L2 packs of G-1 | L1 quad |
            #   L3 quad of G-2.
            NGTOT = nw * NGW
            pos = {}    # window -> psum accumulator for layer 3
            hs = {}     # G -> [4 h tiles]
            p1s_live = {}   # G -> [4 psum tiles] for the quad
            h2s = {}    # (G, band) -> h2 tile

            DR = mybir.MatmulPerfMode.DoubleRowSwInterleave

            def emit_l1_fulls(G, js):
                w, g = divmod(G, NGW)
                if g == 0 and js[0] == 0 and w not in xts:
                    xts[w] = load_xt(w)
                xt, _ = xts[w]
                ps = p1s_live.setdefault(G, [None] * 4)
                KH = K1 // 2
                for j in js:
                    m = 4 * g + j
                    p1 = ps1.tile([128, WN], f32, tag="p1", name=f"p1_{G}_{j}")
                    if m < NM8:
                        # chunks 2,3 first (start), then the fp8 DoubleRow
                        # pair for chunks 0-1 (its weight load hides behind
                        # the k2/k3 streams), then chunks 4,5
                        for k in (2, 3):
                            nc.tensor.matmul(
                                p1[:],
                                w1t_t[m][:, k - 2, :],
                                xt[k >= KH][:, k - KH * (k >= KH), :],
                                start=(k == 2),
                                stop=False,
                            )
                        nc.tensor.matmul(
                            p1[:],
                            w18_t[m][:],
                            xt[2][:],
                            start=False,
                            stop=False,
                            perf_mode=DR,
                        )
                        for k in (4, 5):
                            nc.tensor.matmul(
                                p1[:],
                                w1t_t[m][:, k - 2, :],
                                xt[1][:, k - KH, :],
                                start=False,
                                stop=False,
                            )
                    else:
                        for k in range(K1):
                            hi = k >= KH
                            nc.tensor.matmul(
                                p1[:],
                                w1t_t[m][:, k, :],
                                xt[hi][:, k - KH * hi, :],
                                start=(k == 0),
                                stop=False,
                            )
                    ps[j] = p1

            def emit_l1_quad(G):
                w, g = divmod(G, NGW)
                _, xl = xts[w]
                ps = p1s_live.pop(G)
                for j in range(4):
                    m = 4 * g + j
                    nc.tensor.matmul(
                        ps[j][:],
                        w1l_sb[32 * j:32 * j + KL, m * 128:(m + 1) * 128],
                        xl[32 * j:32 * j + KL, :],
                        start=False,
                        stop=True,
                        tile_position=(32 * j, 0),
                    )
                hts = []
                for j in range(4):
                    m = 4 * g + j
                    h = hpool.tile([128, WN], bf16, tag="h", name=f"h_{G}_{j}")
                    nc.scalar.activation(h[:], ps[j][:], Relu, bias=b1_sb[:, m:m + 1])
                    hts.append(h)
                hs[G] = hts

            def emit_l2_pack(G, half):
                """8 concurrent 32x32 tiles: block rows (2*half, 2*half+1) of
                the 4 m-tiles of group G, into 2 psum banks."""
                w, g = divmod(G, NGW)
                hts = hs[G]
                pk = [
                    ps2.tile([128, WN], f32, tag="p2", name=f"p2_{G}_{half}_{i}")
                    for i in range(2)
                ]
                for c in range(4):
                    for i in range(2):
                        r = 2 * half + i
                        nc.tensor.matmul(
                            pk[i][32 * c:32 * c + 32, :],
                            wb2_sb[32 * r:32 * r + 32,
                                   g * 128 + c * 32:g * 128 + c * 32 + 32],
                            hts[c][32 * r:32 * r + 32, :],
                            start=True,
                            stop=True,
                            tile_position=(32 * r, 32 * c),
                        )
                if half == 1:
                    hs.pop(G)
                for i in range(2):
                    band = 2 * half + i
                    h2 = h2pool.tile([128, WN], bf16, tag="h2",
                                     name=f"h2_{G}_{band}")
                    nc.vector.tensor_scalar(
                        h2[:], pk[i][:], bb_sb[:, 4 * g + band:4 * g + band + 1],
                        0.0, Add, Max,
                    )
                    h2s[(G, band)] = h2

            def emit_l3_quad(G):
                w, g = divmod(G, NGW)
                if g == 0:
                    pos[w] = ps3.tile([128, WN], f32, tag="po", name=f"po_{w}")
                po = pos[w]
                for band in range(NBAND):
                    t = 4 * g + band
                    nc.tensor.matmul(
                        po[32 * band:32 * band + OUT_DIM, :],
                        w3p_sb[:, t * OUT_DIM:(t + 1) * OUT_DIM],
                        h2s.pop((G, band))[:],
                        start=(g == 0),
                        stop=(g == NGW - 1),
                        skip_group_check=True,
                        tile_position=(0, 32 * band),
                    )
                if g == NGW - 1:
                    # band merge: only one PSUM operand per DVE op, so chain
                    # SBUF accumulators across the four bands (+ b3).
                    po = pos.pop(w)
                    t1 = opool.tile([OUT_DIM, WN], f32, tag="t1", name=f"t1_{w}")
                    nc.vector.tensor_scalar_add(t1[:], po[0:OUT_DIM, :], b3_sb[:])
                    t2 = opool.tile([OUT_DIM, WN], f32, tag="t2", name=f"t2_{w}")
                    nc.vector.tensor_tensor(
                        t2[:], t1[:], po[32:32 + OUT_DIM, :], Add
                    )
                    t3 = opool.tile([OUT_DIM, WN], f32, tag="t3", name=f"t3_{w}")
                    nc.vector.tensor_tensor(
                        t3[:], t2[:], po[64:64 + OUT_DIM, :], Add
                    )
                    ot = opool.tile([OUT_DIM, WN], f32, tag="ot", name=f"ot_{w}")
                    nc.vector.tensor_tensor(
                        ot[:], t3[:], po[96:96 + OUT_DIM, :], Add
                    )
                    nc.sync.dma_start(outT[:, w * WN:(w + 1) * WN], ot[:])

            for G in range(NGTOT + 2):
                if G < NGTOT:
                    emit_l1_fulls(G, (0, 1))
                if 1 <= G <= NGTOT:
                    emit_l2_pack(G - 1, 0)
                if G < NGTOT:
                    emit_l1_fulls(G, (2, 3))
                if 1 <= G <= NGTOT:
                    emit_l2_pack(G - 1, 1)
                if G < NGTOT:
                    emit_l1_quad(G)
                if G >= 2:
                    emit_l3_quad(G - 2)

    nc.compile()
    return nc


def _get_program(bc=BC):
    if bc not in _PROGRAM_CACHE:
        _PROGRAM_CACHE[bc] = _build_program(bc)
    return _PROGRAM_CACHE[bc]


def _prep_weights(W1, b1, Wb, bb, W3, b3):
    """Host-side packing of replicated weights into device layouts."""
    W1 = np.asarray(W1, dtype=np.float32)
    Wb = np.asarray(Wb, dtype=np.float32)
    W3 = np.asarray(W3, dtype=np.float32)
    bb = np.asarray(bb, dtype=np.float32)

    # W1T [128, NM, K1, 128]: W1T[p, m, k, c] = W1.T[128k+p, 128m+c] -- the
    # per-partition-contiguous swizzle (1.5 KB DMA runs, per-m-tile DMA
    # granularity). W1L [128, 4096] holds the 16 leftover feature rows
    # replicated at partition offsets 0/32/64/96 for the row-group-packed
    # leftover matmuls.
    W1T = np.ascontiguousarray(
        W1.T[:K1 * 128].reshape(K1, 128, NM, 128).transpose(1, 2, 0, 3)
    ).astype(BF16)
    # W18 [128, NM8, 2, 128]: fp8 copy of chunks 0-1 for m-tiles < NM8,
    # scaled by SW8 (x carries 1/SW8, so products land at scale 1).
    # Stored in DoubleRowSwInterleave layout: flat free dim holds
    # [A127 B127 A126 B126 ... A0 B0] (A/B = chunk0/1 weight columns,
    # reversed order) so the HW weight load reads contiguously.
    W18L = (
        (W1.T[:2 * 128] * SW8).reshape(2, 128, NM, 128)[:, :, :NM8]
        .transpose(1, 2, 0, 3)
    ).astype(E4M3)  # logical [p, m, i, c]
    W18 = np.ascontiguousarray(
        W18L[:, :, :, ::-1].transpose(0, 1, 3, 2)
    ).reshape(128, NM8, 2, 128)
    W1L = np.zeros((128, HIDDEN), dtype=BF16)
    lo = W1.T[K1 * 128:IN_DIM].astype(BF16)
    for j in range(4):
        W1L[32 * j:32 * j + KL] = lo

    # Wb2 [128, NGW*128]: block (4*(4g+c)+r).T at [32r:+32, g*128+c*32:+32].
    Wb2 = np.zeros((128, NGW * 128), dtype=BF16)
    for g in range(NGW):
        for c in range(4):
            for r in range(4):
                blk = Wb[4 * (4 * g + c) + r].T.astype(BF16)  # [k, o]
                Wb2[32 * r:32 * r + 32,
                    g * 128 + c * 32:g * 128 + c * 32 + 32] = blk

    # Pack-output permutation: h2 band tile (g, band) partition 32c+o holds
    # hidden dim 128*(4g+c) + 32*band + o.
    # W3P [128, NM*10]: chunk t=4g+band at cols [t*10:+10]; W3P[p, t*10+o] =
    # W3[o, hid(g, band, p)].
    W3P = np.zeros((128, NM * OUT_DIM), dtype=BF16)
    bbP = np.zeros((128, 128), dtype=np.float32)
    for g in range(NGW):
        for band in range(4):
            t = 4 * g + band
            for c in range(4):
                hid0 = 128 * (4 * g + c) + 32 * band
                W3P[32 * c:32 * c + 32, t * OUT_DIM:(t + 1) * OUT_DIM] = (
                    W3[:, hid0:hid0 + 32].T.astype(BF16)
                )
                bbP[32 * c:32 * c + 32, t] = bb[hid0:hid0 + 32]

    NBC = NM + 128 + 1
    bcat = np.zeros((128, NBC), dtype=np.float32)
    bcat[:, 0:NM] = np.asarray(b1, np.float32).reshape(NM, 128).T
    bcat[:, NM:NM + 128] = bbP
    bcat[0:OUT_DIM, NM + 128] = np.asarray(b3, np.float32)
    return dict(W1T=W1T, W18=W18, W1L=W1L, Wb2=Wb2, W3P=W3P, bcat=bcat)


def _prep_x_shard(x, c, ncores=NCORES, bc=BC):
    xf = np.asarray(x[c * bc:(c + 1) * bc], dtype=np.float32).T  # [784, bc]
    xs = xf.astype(BF16)
    nw = bc // WN
    # xT [128, nw, K1, WN]: xT[p, w, k, b] = x.T[128k+p, 512w+b]
    xT = np.ascontiguousarray(
        xs[:K1 * 128].reshape(K1, 128, nw, WN).transpose(1, 2, 0, 3)
    )
    # X8 [128, nw, 2, WN]: fp8 copy of chunks 0-1, scaled by 1/SW8
    X8 = np.ascontiguousarray(
        (xf[:2 * 128] / SW8).reshape(2, 128, nw, WN).transpose(1, 2, 0, 3)
    ).astype(E4M3)
    xLs = np.zeros((128, bc), dtype=BF16)
    for j in range(4):
        xLs[32 * j:32 * j + KL] = xs[K1 * 128:IN_DIM]
    return xT, X8, xLs


def run(x, W1, b1, Wb, bb, W3, b3, trace=False, tmpdir=None):
    """Run on 8 cores; returns (out [B,10] fp32, BassKernelResults)."""
    from concourse.bass_utils import run_bass_kernel_spmd

    nc = _get_program()
    wmap = _prep_weights(W1, b1, Wb, bb, W3, b3)
    in_maps = []
    for c in range(NCORES):
        m = dict(wmap)
        m["xT"], m["X8"], m["xL"] = _prep_x_shard(np.asarray(x), c)
        in_maps.append(m)

    res = run_bass_kernel_spmd(
        nc, in_maps, core_ids=list(range(NCORES)), trace=trace, tmpdir=tmpdir
    )
    out = np.concatenate(
        [np.asarray(r["outT"]).T for r in res.results], axis=0
    ).astype(np.float32)
    return out, res


def kernel(x, W1, b1, Wb, bb, W3, b3):
    out, _ = run(x, W1, b1, Wb, bb, W3, b3, trace=False)
    return out


# revision 33
# speedup vs baseline: 1.0848x; 1.0057x over previous
"""Trainium2 Bass kernel for BlockDiagMNIST MLP.

Reference computation (all fp32):
    h  = relu(x @ W1.T + b1)          x:[B,784], W1:[4096,784]    -> [B,4096]
    yb = blockdiag(h, Wb)             Wb:[128,32,32] (h2[b, 32n+o] = sum_k h[b,32n+k] Wb[n,o,k])
    h2 = relu(yb + bb)
    out = h2 @ W3.T + b3              W3:[10,4096]                -> [B,10]

Strategy: pure data-parallel over batch (B=32768 -> 4096 rows/core on 8 cores),
weights replicated.  All matmuls in bf16 (fp32 PSUM accumulation, fp32 biases).
On-chip layout is transposed ("hidden on partitions"): we compute
hT = W1 @ x.T per 512-column batch window.

Layer 2 (block-diagonal) runs as packs of eight concurrent 32x32 PE-array
tiles (tile_position row x col grid): tile (32r, 32c) applies one diagonal
block to h-slice [32r:32r+32] of m-tile (4g+c), writing psum partitions
[32c:32c+32].  Two packs (block rows 0,1 then 2,3) cover a 4-m-tile group
using 2 PSUM banks; outputs land hidden-permuted, which the host-side bb/W3
packing compensates.

Layer 3 (M=10) runs as four concurrent column-tiles: K-chunk t accumulates
into psum partitions [32*(t%4) : +10]; a final DVE pass sums the four bands
(cross-quadrant reads) and adds b3.

Host-side prep (free -- not on the device timeline): transpose + bf16-cast of
x and weights, block/bias/W3 permutation packing.
"""

import numpy as np
import ml_dtypes

B = 32768
IN_DIM = 784
HIDDEN = 4096
BLOCK = 32
NUM_BLOCKS = 128
OUT_DIM = 10
NCORES = 8
BC = B // NCORES          # batch rows per core (4096)
WN = 512                  # batch-window columns (one matmul free-dim)
K1 = 6                    # full 128-row K-chunks for layer 1 (features 0..767)
KL = 16                   # leftover K rows (features 768..783), row-group packed
NM = HIDDEN // 128        # 32 hidden tiles per window
NGW = NM // 4             # 8 groups of 4 m-tiles per window
NBAND = 4                 # layer-3 column-tile bands
NM8 = 22                  # m-tiles whose first 2 K-chunks run in fp8 DoubleRow
SW8 = 32.0                # fp8 weight scale (x scaled by 1/SW8: product scale 1)

BF16 = ml_dtypes.bfloat16
E4M3 = ml_dtypes.float8_e4m3   # TRN FP8_EXP4-compatible (max +-240)

_PROGRAM_CACHE = {}


def _build_program(bc=BC):
    """Build (and bacc-compile) the per-core Bass program. bc = batch cols/core."""
    import concourse.mybir as mybir
    import concourse.tile as tile
    from concourse import bacc

    nw = bc // WN
    f32, bf16 = mybir.dt.float32, mybir.dt.bfloat16
    fp8 = mybir.dt.float8e4

    nc = bacc.Bacc("TRN2", target_bir_lowering=False, debug=False)

    # x / W1 stored pre-swizzled for per-partition contiguity (3 KB runs):
    # xT[p, w, k, b], W1T[p, j, k, m]
    xT = nc.dram_tensor("xT", [128, nw, K1, WN], bf16, kind="ExternalInput").ap()
    x8d = nc.dram_tensor("X8", [128, nw, 2, WN], fp8, kind="ExternalInput").ap()
    xL = nc.dram_tensor("xL", [128, bc], bf16, kind="ExternalInput").ap()
    w1t = nc.dram_tensor(
        "W1T", [128, NM, K1, 128], bf16, kind="ExternalInput"
    ).ap()
    w18d = nc.dram_tensor(
        "W18", [128, NM8, 2, 128], fp8, kind="ExternalInput"
    ).ap()
    w1l = nc.dram_tensor("W1L", [128, HIDDEN], bf16, kind="ExternalInput").ap()
    # Wb2: block (4*(4g+c)+r).T at [32r:32r+32, g*128+c*32 : +32]
    wb2 = nc.dram_tensor("Wb2", [128, NGW * 128], bf16, kind="ExternalInput").ap()
    # W3P: chunk t=4g+band -> [128, 10] at cols t*10; rows permuted to match the
    # pack layout (partition 32c+o of band tile <-> hidden 128*(4g+c)+32*band+o)
    w3p = nc.dram_tensor("W3P", [128, NM * OUT_DIM], bf16, kind="ExternalInput").ap()
    # biases packed into one tensor: cols 0..NM-1 = b1, NM..NM+127 = bbP
    # (col NM+4g+band, permuted like the L2 pack output), col NM+128 = b3
    NBC = NM + 128 + 1
    bcat = nc.dram_tensor("bcat", [128, NBC], f32, kind="ExternalInput").ap()
    outT = nc.dram_tensor("outT", [OUT_DIM, bc], f32, kind="ExternalOutput").ap()

    Relu = mybir.ActivationFunctionType.Relu
    Add = mybir.AluOpType.add
    Max = mybir.AluOpType.max

    MB = 4          # W1T column-block = MB m-tiles (DMA granularity for overlap)
    NJ = NM // MB   # 8 column blocks

    with tile.TileContext(nc) as tc:
        with (
            tc.tile_pool(name="const", bufs=1) as cpool,
            tc.tile_pool(name="xin", bufs=3) as xpool,
            tc.tile_pool(name="hbuf", bufs=10) as hpool,
            tc.tile_pool(name="h2buf", bufs=10) as h2pool,
            tc.tile_pool(name="obuf", bufs=4) as opool,
            tc.tile_pool(name="ps1", bufs=5, space="PSUM") as ps1,
            tc.tile_pool(name="ps2", bufs=2, space="PSUM") as ps2,
            tc.tile_pool(name="ps3", bufs=1, space="PSUM") as ps3,
        ):
            # HAM warmup on an un-DMA'd (garbage) SBUF tile: no data
            # dependency, so the PE clock gate starts ramping at t~0 and the
            # warmup stream covers the DMA-launch latency until real x/W1
            # data lands. Values are irrelevant (dummy psum, never read).
            gsb = cpool.tile([128, WN], bf16, name="warm_src")
            nc.vector.memset(gsb[:], 0)
            pw = ps2.tile([65, WN], f32, tag="p2", name="pwarm")
            for _ in range(20):
                nc.tensor.matmul(
                    pw[:, 0:65], gsb[:, 0:65], gsb[:, 0:65],
                    start=True, stop=True,
                )
            for _ in range(12):
                nc.tensor.matmul(
                    pw[:], gsb[:, 0:65], gsb[:],
                    start=True, stop=True,
                )

            def load_xt(w):
                """Per-window x tiles: two k-half DMAs (parallel queues, so the
                first window's data lands sooner) + the leftover rows."""
                KH = K1 // 2
                t8 = xpool.tile([128, 2, WN], fp8, tag="x8", name=f"x8_{w}")
                nc.sync.dma_start(t8[:], x8d[:, w, :, :])
                ta = xpool.tile([128, KH, WN], bf16, tag="xta", name=f"xta_{w}")
                nc.sync.dma_start(ta[:], xT[:, w, 0:KH, :])
                tb = xpool.tile([128, K1 - KH, WN], bf16, tag="xtb", name=f"xtb_{w}")
                nc.sync.dma_start(tb[:], xT[:, w, KH:K1, :])
                tl = xpool.tile([128, WN], bf16, tag="xl", name=f"xl_{w}")
                if w > 0:
                    nc.sync.dma_start(tl[:], xL[:, w * WN:(w + 1) * WN])
                return (ta, tb, t8), tl

            # Window-0 x tile before everything else so PE starts early.
            xts = {0: load_xt(0)}

            # Small constants (ACT/DVE need them by the first relu).
            bc_sb = cpool.tile([128, NBC], f32)
            nc.sync.dma_start(bc_sb[:], bcat)
            b1_sb = bc_sb[:, 0:NM]
            bb_sb = bc_sb[:, NM:NM + 128]
            b3_sb = bc_sb[0:OUT_DIM, NM + 128:NBC]

            # W1T as NM per-m-tile DMAs [128, K1, 128] (196 KB each): the
            # startup-critical chain is just x window 0 + m-tile 0's weights,
            # so real matmuls start as soon as ~700 KB has landed. The
            # leftover/L2/L3 weights are split per group and interleaved so
            # each group's full weight set arrives just ahead of its compute.
            w1l_sb = cpool.tile([128, HIDDEN], bf16)
            wb2_sb = cpool.tile([128, NGW * 128], bf16)
            w3p_sb = cpool.tile([128, NM * OUT_DIM], bf16)
            # DMA packet discipline: keep per-partition contiguous runs >=1KB.
            # w18 loads per group of 4 m-tiles (1KB/partition), w1l per group
            # (1KB/partition); wb2 (2KB/part) and w3p (640B/part) load whole.
            w18_sb = cpool.tile([128, NM8, 2, 128], fp8, name="w18")
            w18_t = [
                w18_sb[:, m, :, :] for m in range(NM8)
            ]
            nc.sync.dma_start(w18_sb[:, 0:4, :, :], w18d[:, 0:4, :, :])
            w1t_t = [None] * NM
            for m in range(NM):
                if m < NM8:
                    # fp8 m-tile: bf16 weights only for chunks 2..5
                    t = cpool.tile([128, K1 - 2, 128], bf16, name=f"w1m_{m}")
                    nc.sync.dma_start(t[:], w1t[:, m, 2:K1, :])
                else:
                    t = cpool.tile([128, K1, 128], bf16, name=f"w1m_{m}")
                    nc.sync.dma_start(t[:], w1t[:, m, :, :])
                w1t_t[m] = t
                if m == 3:
                    # window-0 leftover rows (needed at group 0's quad)
                    nc.sync.dma_start(xts[0][1][:], xL[:, 0:WN])
                if m % 4 == 3:
                    g = m // 4
                    nc.sync.dma_start(
                        w1l_sb[:, g * WN:(g + 1) * WN],
                        w1l[:, g * WN:(g + 1) * WN],
                    )
                    g8 = g + 1
                    if g8 * 4 < NM8:
                        hi = min((g8 + 1) * 4, NM8)
                        nc.sync.dma_start(
                            w18_sb[:, g8 * 4:hi, :, :], w18d[:, g8 * 4:hi, :, :]
                        )
                    if m == 3:
                        # L2/L3 weights: small heads arrive before the first
                        # packs; the bulk rides one packet-efficient DMA.
                        nc.sync.dma_start(
                            wb2_sb[:, 0:2 * 128], wb2[:, 0:2 * 128]
                        )
                        nc.sync.dma_start(w3p_sb[:], w3p)
                    if m == 7:
                        nc.sync.dma_start(
                            wb2_sb[:, 2 * 128:], wb2[:, 2 * 128:]
                        )

            # Software pipeline over G (global group index = window*NGW + g):
            #   L1(G) fulls interleaved with L2 packs of G-1 | L1 quad |
            #   L3 quad of G-2.
            NGTOT = nw * NGW
            pos = {}    # window -> psum accumulator for layer 3
            hs = {}     # G -> [4 h tiles]
            p1s_live = {}   # G -> [4 psum tiles] for the quad
            h2s = {}    # (G, band) -> h2 tile

            DR = mybir.MatmulPerfMode.DoubleRowSwInterleave

            def emit_l1_fulls(G, js):
                w, g = divmod(G, NGW)
                if g == 0 and js[0] == 0 and w not in xts:
                    xts[w] = load_xt(w)
                xt, _ = xts[w]
                ps = p1s_live.setdefault(G, [None] * 4)
                KH = K1 // 2
                for j in js:
                    m = 4 * g + j
                    p1 = ps1.tile([128, WN], f32, tag="p1", name=f"p1_{G}_{j}")
                    if m < NM8:
                        # chunks 2,3 first (start), then the fp8 DoubleRow
                        # pair for chunks 0-1 (its weight load hides behind
                        # the k2/k3 streams), then chunks 4,5
                        for k in (2, 3):
                            nc.tensor.matmul(
                                p1[:],
                                w1t_t[m][:, k - 2, :],
                                xt[k >= KH][:, k - KH * (k >= KH), :],
                                start=(k == 2),
                                stop=False,
                            )
                        nc.tensor.matmul(
                            p1[:],
                            w18_t[m][:],
                            xt[2][:],
                            start=False,
                            stop=False,
                            perf_mode=DR,
                        )
                        for k in (4, 5):
                            nc.tensor.matmul(
                                p1[:],
                                w1t_t[m][:, k - 2, :],
                                xt[1][:, k - KH, :],
                                start=False,
                                stop=False,
                            )
                    else:
                        for k in range(K1):
                            hi = k >= KH
                            nc.tensor.matmul(
                                p1[:],
                                w1t_t[m][:, k, :],
                                xt[hi][:, k - KH * hi, :],
                                start=(k == 0),
                                stop=False,
                            )
                    ps[j] = p1

            def emit_l1_quad(G):
                w, g = divmod(G, NGW)
                _, xl = xts[w]
                ps = p1s_live.pop(G)
                for j in range(4):
                    m = 4 * g + j
                    nc.tensor.matmul(
                        ps[j][:],
                        w1l_sb[32 * j:32 * j + KL, m * 128:(m + 1) * 128],
                        xl[32 * j:32 * j + KL, :],
                        start=False,
                        stop=True,
                        tile_position=(32 * j, 0),
                    )
                hts = []
                for j in range(4):
                    m = 4 * g + j
                    h = hpool.tile([128, WN], bf16, tag="h", name=f"h_{G}_{j}")
                    nc.scalar.activation(h[:], ps[j][:], Relu, bias=b1_sb[:, m:m + 1])
                    hts.append(h)
                hs[G] = hts

            def emit_l2_pack(G, half):
                """8 concurrent 32x32 tiles: block rows (2*half, 2*half+1) of
                the 4 m-tiles of group G, into 2 psum banks."""
                w, g = divmod(G, NGW)
                hts = hs[G]
                pk = [
                    ps2.tile([128, WN], f32, tag="p2", name=f"p2_{G}_{half}_{i}")
                    for i in range(2)
                ]
                for c in range(4):
                    for i in range(2):
                        r = 2 * half + i
                        nc.tensor.matmul(
                            pk[i][32 * c:32 * c + 32, :],
                            wb2_sb[32 * r:32 * r + 32,
                                   g * 128 + c * 32:g * 128 + c * 32 + 32],
                            hts[c][32 * r:32 * r + 32, :],
                            start=True,
                            stop=True,
                            tile_position=(32 * r, 32 * c),
                        )
                if half == 1:
                    hs.pop(G)
                for i in range(2):
                    band = 2 * half + i
                    h2 = h2pool.tile([128, WN], bf16, tag="h2",
                                     name=f"h2_{G}_{band}")
                    nc.vector.tensor_scalar(
                        h2[:], pk[i][:], bb_sb[:, 4 * g + band:4 * g + band + 1],
                        0.0, Add, Max,
                    )
                    h2s[(G, band)] = h2

            def emit_l3_quad(G):
                w, g = divmod(G, NGW)
                if g == 0:
                    pos[w] = ps3.tile([128, WN], f32, tag="po", name=f"po_{w}")
                po = pos[w]
                for band in range(NBAND):
                    t = 4 * g + band
                    nc.tensor.matmul(
                        po[32 * band:32 * band + OUT_DIM, :],
                        w3p_sb[:, t * OUT_DIM:(t + 1) * OUT_DIM],
                        h2s.pop((G, band))[:],
                        start=(g == 0),
                        stop=(g == NGW - 1),
                        skip_group_check=True,
                        tile_position=(0, 32 * band),
                    )
                if g == NGW - 1:
                    # band merge: only one PSUM operand per DVE op, so chain
                    # SBUF accumulators across the four bands (+ b3).
                    po = pos.pop(w)
                    t1 = opool.tile([OUT_DIM, WN], f32, tag="t1", name=f"t1_{w}")
                    nc.vector.tensor_scalar_add(t1[:], po[0:OUT_DIM, :], b3_sb[:])
                    t2 = opool.tile([OUT_DIM, WN], f32, tag="t2", name=f"t2_{w}")
                    nc.vector.tensor_tensor(
                        t2[:], t1[:], po[32:32 + OUT_DIM, :], Add
                    )
                    t3 = opool.tile([OUT_DIM, WN], f32, tag="t3", name=f"t3_{w}")
                    nc.vector.tensor_tensor(
                        t3[:], t2[:], po[64:64 + OUT_DIM, :], Add
                    )
                    ot = opool.tile([OUT_DIM, WN], f32, tag="ot", name=f"ot_{w}")
                    nc.vector.tensor_tensor(
                        ot[:], t3[:], po[96:96 + OUT_DIM, :], Add
                    )
                    nc.sync.dma_start(outT[:, w * WN:(w + 1) * WN], ot[:])

            for G in range(NGTOT + 2):
                if G < NGTOT:
                    emit_l1_fulls(G, (0, 1))
                if 1 <= G <= NGTOT:
                    emit_l2_pack(G - 1, 0)
                if G < NGTOT:
                    emit_l1_fulls(G, (2, 3))
                if 1 <= G <= NGTOT:
                    emit_l2_pack(G - 1, 1)
                if G < NGTOT:
                    emit_l1_quad(G)
                if G >= 2:
                    emit_l3_quad(G - 2)

    nc.compile()
    return nc


def _get_program(bc=BC):
    if bc not in _PROGRAM_CACHE:
        _PROGRAM_CACHE[bc] = _build_program(bc)
    return _PROGRAM_CACHE[bc]


def _prep_weights(W1, b1, Wb, bb, W3, b3):
    """Host-side packing of replicated weights into device layouts."""
    W1 = np.asarray(W1, dtype=np.float32)
    Wb = np.asarray(Wb, dtype=np.float32)
    W3 = np.asarray(W3, dtype=np.float32)
    bb = np.asarray(bb, dtype=np.float32)

    # W1T [128, NM, K1, 128]: W1T[p, m, k, c] = W1.T[128k+p, 128m+c] -- the
    # per-partition-contiguous swizzle (1.5 KB DMA runs, per-m-tile DMA
    # granularity). W1L [128, 4096] holds the 16 leftover feature rows
    # replicated at partition offsets 0/32/64/96 for the row-group-packed
    # leftover matmuls.
    W1T = np.ascontiguousarray(
        W1.T[:K1 * 128].reshape(K1, 128, NM, 128).transpose(1, 2, 0, 3)
    ).astype(BF16)
    # W18 [128, NM8, 2, 128]: fp8 copy of chunks 0-1 for m-tiles < NM8,
    # scaled by SW8 (x carries 1/SW8, so products land at scale 1).
    # Stored in DoubleRowSwInterleave layout: flat free dim holds
    # [A127 B127 A126 B126 ... A0 B0] (A/B = chunk0/1 weight columns,
    # reversed order) so the HW weight load reads contiguously.
    W18L = (
        (W1.T[:2 * 128] * SW8).reshape(2, 128, NM, 128)[:, :, :NM8]
        .transpose(1, 2, 0, 3)
    ).astype(E4M3)  # logical [p, m, i, c]
    W18 = np.ascontiguousarray(
        W18L[:, :, :, ::-1].transpose(0, 1, 3, 2)
    ).reshape(128, NM8, 2, 128)
    W1L = np.zeros((128, HIDDEN), dtype=BF16)
    lo = W1.T[K1 * 128:IN_DIM].astype(BF16)
    for j in range(4):
        W1L[32 * j:32 * j + KL] = lo

    # Wb2 [128, NGW*128]: block (4*(4g+c)+r).T at [32r:+32, g*128+c*32:+32].
    Wb2 = np.zeros((128, NGW * 128), dtype=BF16)
    for g in range(NGW):
        for c in range(4):
            for r in range(4):
                blk = Wb[4 * (4 * g + c) + r].T.astype(BF16)  # [k, o]
                Wb2[32 * r:32 * r + 32,
                    g * 128 + c * 32:g * 128 + c * 32 + 32] = blk

    # Pack-output permutation: h2 band tile (g, band) partition 32c+o holds
    # hidden dim 128*(4g+c) + 32*band + o.
    # W3P [128, NM*10]: chunk t=4g+band at cols [t*10:+10]; W3P[p, t*10+o] =
    # W3[o, hid(g, band, p)].
    W3P = np.zeros((128, NM * OUT_DIM), dtype=BF16)
    bbP = np.zeros((128, 128), dtype=np.float32)
    for g in range(NGW):
        for band in range(4):
            t = 4 * g + band
            for c in range(4):
                hid0 = 128 * (4 * g + c) + 32 * band
                W3P[32 * c:32 * c + 32, t * OUT_DIM:(t + 1) * OUT_DIM] = (
                    W3[:, hid0:hid0 + 32].T.astype(BF16)
                )
                bbP[32 * c:32 * c + 32, t] = bb[hid0:hid0 + 32]

    NBC = NM + 128 + 1
    bcat = np.zeros((128, NBC), dtype=np.float32)
    bcat[:, 0:NM] = np.asarray(b1, np.float32).reshape(NM, 128).T
    bcat[:, NM:NM + 128] = bbP
    bcat[0:OUT_DIM, NM + 128] = np.asarray(b3, np.float32)
    return dict(W1T=W1T, W18=W18, W1L=W1L, Wb2=Wb2, W3P=W3P, bcat=bcat)


def _prep_x_shard(x, c, ncores=NCORES, bc=BC):
    xf = np.asarray(x[c * bc:(c + 1) * bc], dtype=np.float32).T  # [784, bc]
    xs = xf.astype(BF16)
    nw = bc // WN
    # xT [128, nw, K1, WN]: xT[p, w, k, b] = x.T[128k+p, 512w+b]
    xT = np.ascontiguousarray(
        xs[:K1 * 128].reshape(K1, 128, nw, WN).transpose(1, 2, 0, 3)
    )
    # X8 [128, nw, 2, WN]: fp8 copy of chunks 0-1, scaled by 1/SW8
    X8 = np.ascontiguousarray(
        (xf[:2 * 128] / SW8).reshape(2, 128, nw, WN).transpose(1, 2, 0, 3)
    ).astype(E4M3)
    xLs = np.zeros((128, bc), dtype=BF16)
    for j in range(4):
        xLs[32 * j:32 * j + KL] = xs[K1 * 128:IN_DIM]
    return xT, X8, xLs


def run(x, W1, b1, Wb, bb, W3, b3, trace=False, tmpdir=None):
    """Run on 8 cores; returns (out [B,10] fp32, BassKernelResults)."""
    from concourse.bass_utils import run_bass_kernel_spmd

    nc = _get_program()
    wmap = _prep_weights(W1, b1, Wb, bb, W3, b3)
    in_maps = []
    for c in range(NCORES):
        m = dict(wmap)
        m["xT"], m["X8"], m["xL"] = _prep_x_shard(np.asarray(x), c)
        in_maps.append(m)

    res = run_bass_kernel_spmd(
        nc, in_maps, core_ids=list(range(NCORES)), trace=trace, tmpdir=tmpdir
    )
    out = np.concatenate(
        [np.asarray(r["outT"]).T for r in res.results], axis=0
    ).astype(np.float32)
    return out, res


def kernel(x, W1, b1, Wb, bb, W3, b3):
    out, _ = run(x, W1, b1, Wb, bb, W3, b3, trace=False)
    return out
